# revision 10
# baseline (speedup 1.0000x reference)
"""Trainium2 Bass kernel for nn_DenTargetTransformerConv (GNN message passing).

Strategy (graph/data parallel, dst-owner sharding across 8 NeuronCores):
  - Nodes are partitioned by dst-id range; each core owns N/8 nodes and all
    edges whose dst falls in its range. Cores are fully independent (the
    "halo exchange" of src features is materialized host-side as per-core
    compacted per-edge tables; the device streams them contiguously).
  - Per core, own nodes are sorted by in-degree and packed into groups of
    128 (SBUF partition dim). Every node in group g gets K[g] edge slots
    (K[g] = max degree in that group position across all cores, so the 8
    cores share one compiled program).
  - All per-edge data lives in fp16 with (d, h)-interleaved head layout so
    every DVE op is a dense step-1 16-bit op (2x perf mode). The edge table
    is stored partition-major in DRAM, so each run is one big contiguous
    dma_start per partition (no gather descriptors).
  - Reductions avoid tensor_reduce (always 1x on DVE) where they are large:
    the D-reduction of scores and the K-reduction of the weighted values are
    log2 trees of 2x tensor_tensor adds. Padded slots carry q=v=0; their
    exp(0-2) contribution to the softmax denominator is removed with a
    host-staged pad-count correction instead of a mask multiply.
  - Runs are software-pipelined (post-exp work of run r issues after the
    pre-exp work of run r+1) so the ACT-engine exp never stalls the DVE.
    The node phase (gate/LayerNorm/PReLU) runs in two interleaved chunks so
    its ACT broadcasts and output DMA overlap DVE work.
"""

import numpy as np

import concourse.bacc as bacc
import concourse.bass as bass
import concourse.tile as tile
from concourse import mybir
from concourse.bass_utils import run_bass_kernel_spmd

F32 = mybir.dt.float32
F16 = mybir.dt.float16
AX = mybir.AxisListType
ALU = mybir.AluOpType
ACTF = mybir.ActivationFunctionType

P = 128
NCORES = 8
HD = 64          # H * D
H, D = 4, 16
IN_F = 64

RUNC = 80        # max slot-columns per merged compute run
GP_MAXRK = 36    # runs this size or smaller execute on the GpSimd engine

# fp16 value the ACT exp produces for a fully-padded slot (exp(0*0.25 - 2))
EXPV = float(np.float32(np.float16(np.exp(-2.0))))


def _perm_dh(m):
    """Permute the last hd axis from (h, d) to (d, h) order."""
    s = m.shape[:-1]
    return m.reshape(*s, H, D).swapaxes(-1, -2).reshape(*s, HD)


# ----------------------------------------------------------------- host prep

def _plan(q_src, v_src, feat, src, dst, ncores):
    n = feat.shape[0]
    npc = n // ncores
    ngrp = (npc + P - 1) // P
    grid = ngrp * P
    ndum = grid - npc

    q2 = _perm_dh(np.asarray(q_src, np.float32).reshape(n, HD))
    v2 = _perm_dh(np.asarray(v_src, np.float32).reshape(n, HD))
    qv = np.concatenate([q2, v2], axis=1).astype(np.float16)   # [n, 128]

    src = np.asarray(src).astype(np.int64)
    dst = np.asarray(dst).astype(np.int64)
    order = np.argsort(dst, kind="stable")
    dst_s, src_s = dst[order], src[order]
    bounds = np.searchsorted(dst_s, np.arange(ncores + 1) * npc)

    cores = []
    gmax = np.zeros((ncores, ngrp), np.int64)
    gdegs = []
    for c in range(ncores):
        lo, hi = bounds[c], bounds[c + 1]
        dstL = dst_s[lo:hi] - c * npc          # ascending
        srcL = src_s[lo:hi]
        deg = np.bincount(dstL, minlength=npc)
        starts = np.concatenate([[0], np.cumsum(deg)])
        rank = np.arange(len(dstL)) - starts[dstL]
        perm = np.argsort(deg, kind="stable")  # ascending degree
        pos_of = np.empty(npc, np.int64)
        pos_of[perm] = ndum + np.arange(npc)
        gd = np.zeros(grid, np.int64)
        gd[ndum:] = deg[perm]
        gmax[c] = gd.reshape(ngrp, P).max(1)
        gdegs.append(gd)
        cores.append(dict(dstL=dstL, srcL=srcL, rank=rank, perm=perm,
                          pos_of=pos_of))

    K = np.maximum(gmax.max(0), 2)             # shared per-group slot count
    colbase = np.concatenate([[0], np.cumsum(K)]).astype(np.int64)
    totc = int(colbase[-1])

    # Per-core edge tables, partition-major: tab[p, col, :] is the qv row of
    # the edge in slot (group g, partition p, rank k), col = colbase[g] + k.
    # negpad removes the padded slots' exp(-2) from the softmax denominator
    # (and folds in the 1e-9 epsilon).
    per_core = []
    for c in range(ncores):
        cd = cores[c]
        pos_e = cd["pos_of"][cd["dstL"]]       # grid position of each edge
        g_e = pos_e // P
        p_e = pos_e % P
        col_e = colbase[g_e] + cd["rank"]
        tab = np.zeros((P, totc, 2 * HD), np.float16)
        tab[p_e, col_e] = qv[cd["srcL"]]
        npad = (K[None, :] - gdegs[c].reshape(ngrp, P).T)      # [P, ngrp]
        negpad = np.repeat((-npad * EXPV + 1e-9).astype(np.float32),
                           H, axis=1)                          # [P, ngrp*H]
        per_core.append(dict(tab=tab.reshape(P, totc * 2 * HD),
                             negpad=negpad))

    # featT with ones row, per core, grid-permuted: [IN_F+1, grid] fp16
    featTs = []
    feat = np.asarray(feat, np.float32)
    for c in range(ncores):
        ft = np.zeros((IN_F + 1, grid), np.float16)
        ft[IN_F, :] = 1.0
        perm = cores[c]["perm"]
        ft[:IN_F, ndum:] = feat[c * npc + perm].T.astype(np.float16)
        featTs.append(ft)

    # Merge consecutive equal-K groups into runs of <= RUNC slot-columns.
    runs = []
    g = 0
    while g < ngrp:
        k = int(K[g])
        ge = g + 1
        while ge < ngrp and int(K[ge]) == k and (ge - g + 1) * k <= RUNC:
            ge += 1
        runs.append((g, ge, k))
        g = ge

    return dict(n=n, npc=npc, ngrp=ngrp, grid=grid, ndum=ndum, K=K,
                colbase=colbase, totc=totc, runs=runs,
                cores=cores, per_core=per_core, featTs=featTs)


# ------------------------------------------------------------- device build

def _ap(view, off, dims):
    """AP over a tile view's buffer: partition dim kept, free dims replaced."""
    return bass.AP(tensor=view.tensor, offset=view.offset + off,
                   ap=[view.ap[0]] + dims)


def _build_nc(plan, ncores):
    ngrp, totc, runs = plan["ngrp"], plan["totc"], plan["runs"]
    grid = plan["grid"]
    colbase = plan["colbase"]
    NG = ngrp

    nc = bacc.Bacc("TRN2", target_bir_lowering=False, debug=False,
                   num_devices=ncores)

    featT_d = nc.dram_tensor("featT", [IN_F + 1, grid], F16,
                             kind="ExternalInput").ap()
    tab_d = nc.dram_tensor("tab", [P, totc * 2 * HD], F16,
                           kind="ExternalInput").ap()
    negpad_d = nc.dram_tensor("negpad", [P, NG * H], F32,
                              kind="ExternalInput").ap()
    # combined node linear weights: cols 0:64 = Wskip|bskip, 64:128 = Wk|bk
    wks_d = nc.dram_tensor("wks", [IN_F + 1, 2 * HD], F16,
                           kind="ExternalInput").ap()
    # fp16 params: [wg_skip (64) | wg_rst (64) | gamma (64) | beta (64)]
    par16_d = nc.dram_tensor("par16", [1, 4 * HD], F16,
                             kind="ExternalInput").ap()
    # fp32 params: [bgate, prelu_a, ln_eps, -2.0]
    par32_d = nc.dram_tensor("par32", [1, 4], F32, kind="ExternalInput").ap()
    out_d = nc.dram_tensor("out", [P, ngrp * 2 * HD], F16,
                           kind="ExternalOutput").ap()

    GRC = min(GP_MAXRK, RUNC)
    with tile.TileContext(nc) as tc:
        with (
            tc.tile_pool(name="singles", bufs=1) as singles,
            tc.tile_pool(name="psum", bufs=2, space="PSUM") as psum,
            tc.tile_pool(name="qvp", bufs=3) as qvp,
            tc.tile_pool(name="scr", bufs=3) as scr,
            tc.tile_pool(name="t1p", bufs=2) as t1p,
            tc.tile_pool(name="t2p", bufs=2) as t2p,
            tc.tile_pool(name="t3p", bufs=2) as t3p,
            tc.tile_pool(name="exp", bufs=4) as exsp,
            tc.tile_pool(name="qvg", bufs=2) as qvg,
            tc.tile_pool(name="scrg", bufs=2) as scrg,
            tc.tile_pool(name="t1g", bufs=2) as t1g,
            tc.tile_pool(name="t2g", bufs=2) as t2g,
            tc.tile_pool(name="t3g", bufs=2) as t3g,
            tc.tile_pool(name="exg", bufs=8) as exg,
        ):
            # ---- static loads (wks first: matmuls need it + featT chunk)
            wks_sb = singles.tile([IN_F + 1, 2 * HD], F16)
            nc.sync.dma_start(out=wks_sb[:], in_=wks_d[:])
            featT = singles.tile([IN_F + 1, grid], F16)
            FCH = 13 * P
            for f0 in range(0, grid, FCH):
                f1 = min(grid, f0 + FCH)
                nc.sync.dma_start(out=featT[:, f0:f1], in_=featT_d[:, f0:f1])
            p16 = singles.tile([P, 4 * HD], F16)
            nc.gpsimd.dma_start(
                out=p16[:],
                in_=bass.AP(tensor=par16_d.tensor, offset=par16_d.offset,
                            ap=[[0, P], [1, 4 * HD]]))
            p32 = singles.tile([P, 4], F32)
            nc.gpsimd.dma_start(
                out=p32[:],
                in_=bass.AP(tensor=par32_d.tensor, offset=par32_d.offset,
                            ap=[[0, P], [1, 4]]))
            negpad_sb = singles.tile([P, NG * H], F32)
            nc.sync.dma_start(out=negpad_sb[:], in_=negpad_d[:])
            bg = p32[:, 0:1]
            pa = p32[:, 1:2]
            eps_t = p32[:, 2:3]
            nbias = p32[:, 3:4]          # -2.0 shift for exp

            # ksk: per group g, cols [g*128, g*128+64) = skip,
            # [g*128+64, (g+1)*128) = k16 (later overwritten by rst).
            ksk = singles.tile([P, NG * 2 * HD], F16)
            den = singles.tile([P, NG * H], F32)

            # ---- node linears on PE, 4 groups per PSUM bank
            g = 0
            while g < NG:
                nb = min(4, NG - g)
                pk = psum.tile([P, 512], F32, tag="mm")
                for j in range(nb):
                    nc.tensor.matmul(out=pk[:, j * 128:(j + 1) * 128],
                                     lhsT=featT[:, (g + j) * P:(g + j + 1) * P],
                                     rhs=wks_sb[:],
                                     start=True, stop=True)
                nc.scalar.activation(out=ksk[:, g * 128:(g + nb) * 128],
                                     in_=pk[:, :nb * 128], func=ACTF.Copy)
                g += nb

            # ---- edge phase: big runs on DVE (software-pipelined around the
            # ACT exp), small runs on the otherwise-idle GpSimd engine with
            # dedicated pools so the two streams never share buffers.
            def emit_pre(g0, g1, K, eng, pq, ps, p1, p2, p3, pe, cap):
                R = g1 - g0
                RK = R * K
                c0 = int(colbase[g0])
                qv_t = pq.tile([P, cap * 2 * HD], F16, tag="qv")
                nc.sync.dma_start(out=qv_t[:, :RK * 2 * HD],
                                  in_=tab_d[:, c0 * 2 * HD:(c0 + RK) * 2 * HD])
                qv0 = qv_t[:, 0:1]

                # prod[p, (r,k), dh] = q * k_dst  (k16 bcast over k slots)
                prod = ps.tile([P, cap * HD], F16, tag="scr")
                pr0 = prod[:, 0:1]
                eng.tensor_tensor(
                    out=_ap(pr0, 0, [[HD * K, R], [HD, K], [1, HD]]),
                    in0=_ap(qv0, 0, [[2 * HD * K, R], [2 * HD, K], [1, HD]]),
                    in1=_ap(ksk[:, 0:1], g0 * 2 * HD + HD,
                            [[2 * HD, R], [0, K], [1, HD]]),
                    op=ALU.mult)

                # score tree over d: prod [p, rk, d16, h] -> a [p, rk, h]
                t1 = p1.tile([P, cap * 32], F16, tag="t1")
                eng.tensor_tensor(
                    out=_ap(t1[:, 0:1], 0, [[32, RK], [H, 8], [1, H]]),
                    in0=_ap(pr0, 0, [[HD, RK], [H, 8], [1, H]]),
                    in1=_ap(pr0, 32, [[HD, RK], [H, 8], [1, H]]),
                    op=ALU.add)
                t2 = p2.tile([P, cap * 16], F16, tag="t2")
                eng.tensor_tensor(
                    out=_ap(t2[:, 0:1], 0, [[16, RK], [H, 4], [1, H]]),
                    in0=_ap(t1[:, 0:1], 0, [[32, RK], [H, 4], [1, H]]),
                    in1=_ap(t1[:, 0:1], 16, [[32, RK], [H, 4], [1, H]]),
                    op=ALU.add)
                t3 = p3.tile([P, cap * 8], F16, tag="t3")
                eng.tensor_tensor(
                    out=_ap(t3[:, 0:1], 0, [[8, RK], [H, 2], [1, H]]),
                    in0=_ap(t2[:, 0:1], 0, [[16, RK], [H, 2], [1, H]]),
                    in1=_ap(t2[:, 0:1], 8, [[16, RK], [H, 2], [1, H]]),
                    op=ALU.add)
                ex = pe.tile([P, cap * H], F16, tag="ex")
                eng.tensor_tensor(
                    out=_ap(ex[:, 0:1], 0, [[H, RK], [1, H]]),
                    in0=_ap(t3[:, 0:1], 0, [[2 * H, RK], [1, H]]),
                    in1=_ap(t3[:, 0:1], H, [[2 * H, RK], [1, H]]),
                    op=ALU.add)

                # ex = exp(a/4 - 2) (ACT; padded slots give exp(-2), removed
                # from the denominator via negpad)
                exf = ex[:, :RK * H]
                nc.scalar.activation(out=exf, in_=exf, func=ACTF.Exp,
                                     scale=0.25, bias=nbias)
                return qv_t, ex

            def emit_den(g0, g1, K, ex):
                R = g1 - g0
                nc.vector.tensor_reduce(
                    out=_ap(den[:, 0:1], g0 * H, [[H, R], [1, H]]),
                    in_=_ap(ex[:, 0:1], 0, [[K * H, R], [1, H], [H, K]]),
                    axis=AX.X, op=ALU.add)

            def emit_post(g0, g1, K, qv_t, ex, eng, ps, cap):
                R = g1 - g0
                RK = R * K
                qv0 = qv_t[:, 0:1]
                # w[p, rk, d, h] = v * ex (bcast over d)
                w_t = ps.tile([P, cap * HD], F16, tag="scr")
                w0 = w_t[:, 0:1]
                eng.tensor_tensor(
                    out=_ap(w0, 0, [[HD, RK], [H, D], [1, H]]),
                    in0=_ap(qv0, HD, [[2 * HD, RK], [H, D], [1, H]]),
                    in1=_ap(ex[:, 0:1], 0, [[H, RK], [0, D], [1, H]]),
                    op=ALU.mult)

                # agg tree over k -> rst slot of ksk (fp16)
                klen = K
                while klen > 2:
                    h1 = klen // 2
                    eng.tensor_tensor(
                        out=_ap(w0, 0, [[K * HD, R], [HD, h1], [1, HD]]),
                        in0=_ap(w0, 0, [[K * HD, R], [HD, h1], [1, HD]]),
                        in1=_ap(w0, (klen - h1) * HD,
                                [[K * HD, R], [HD, h1], [1, HD]]),
                        op=ALU.add)
                    klen = h1 + (klen & 1)
                eng.tensor_tensor(
                    out=_ap(ksk[:, 0:1], g0 * 2 * HD + HD,
                            [[2 * HD, R], [1, HD]]),
                    in0=_ap(w0, 0, [[K * HD, R], [1, HD]]),
                    in1=_ap(w0, HD, [[K * HD, R], [1, HD]]),
                    op=ALU.add)

            dve_pools = (nc.vector, qvp, scr, t1p, t2p, t3p, exsp, RUNC)
            gp_pools = (nc.gpsimd, qvg, scrg, t1g, t2g, t3g, exg, GRC)
            gp_dens = []
            pend = None
            for (g0, g1, K) in runs:
                R = g1 - g0
                if R * K <= GP_MAXRK:
                    eng, pq, ps, p1, p2, p3, pe, cap = gp_pools
                    qv_t, ex = emit_pre(g0, g1, K, eng, pq, ps, p1, p2, p3,
                                        pe, cap)
                    emit_post(g0, g1, K, qv_t, ex, eng, ps, cap)
                    gp_dens.append((g0, g1, K, ex))
                else:
                    eng, pq, ps, p1, p2, p3, pe, cap = dve_pools
                    pre = emit_pre(g0, g1, K, eng, pq, ps, p1, p2, p3, pe, cap)
                    if pend is not None:
                        emit_den(pend[0], pend[1], pend[2], pend[4])
                        emit_post(*pend, nc.vector, scr, RUNC)
                    pend = (g0, g1, K) + pre
            if pend is not None:
                emit_den(pend[0], pend[1], pend[2], pend[4])
                emit_post(*pend, nc.vector, scr, RUNC)
            for (g0, g1, K, ex) in gp_dens:
                emit_den(g0, g1, K, ex)

            # ---- node phase: two interleaved chunks of groups
            kv = ksk[:, 0:1]

            def node_ops(lo, hi):
                NGc = hi - lo
                dsl = den[:, lo * H:hi * H]
                nsl = negpad_sb[:, lo * H:hi * H]
                d16 = singles.tile([P, NGc * H], F16)
                gl = singles.tile([P, NGc], F32)
                g16 = singles.tile([P, NGc], F16)
                # bc is reused for the three sequential hd-broadcasts
                # (gate, mean, rstd); dif doubles as the square buffer.
                bc = singles.tile([P, NGc * HD], F16)
                gb = bc
                mub = bc
                rb = bc
                dif = singles.tile([P, NGc * HD], F16)
                sq = dif
                mu = singles.tile([P, NGc], F32)
                vs = singles.tile([P, NGc], F32)
                rstf = _ap(kv, lo * 2 * HD + HD, [[2 * HD, NGc], [1, HD]])
                sksl = _ap(kv, lo * 2 * HD, [[2 * HD, NGc], [1, HD]])
                dif3 = dif[:].rearrange("p (c f) -> p c f", f=HD)
                mub3 = mub[:].rearrange("p (c f) -> p c f", f=HD)
                sq3 = sq[:].rearrange("p (c f) -> p c f", f=HD)
                rb3 = rb[:].rearrange("p (c f) -> p c f", f=HD)
                zt = qvp.tile([P, RUNC * 2 * HD], F16, tag="qv")
                ops = [
                    # dinv = 1 / (den - npad*e^-2 + 1e-9), as fp16
                    lambda: nc.vector.tensor_tensor(
                        out=dsl, in0=dsl, in1=nsl, op=ALU.add),
                    lambda: nc.vector.reciprocal(out=dsl, in_=dsl),
                    lambda: nc.scalar.activation(out=d16[:], in_=dsl,
                                                 func=ACTF.Copy),
                    # rst = agg * dinv
                    lambda: nc.vector.tensor_tensor(
                        out=_ap(kv, lo * 2 * HD + HD,
                                [[2 * HD, NGc], [H, D], [1, H]]),
                        in0=_ap(kv, lo * 2 * HD + HD,
                                [[2 * HD, NGc], [H, D], [1, H]]),
                        in1=_ap(d16[:, 0:1], 0, [[H, NGc], [0, D], [1, H]]),
                        op=ALU.mult),
                    # gate logit z = sum over 128 of [skip|rst]*[wgs|wgr]
                    lambda: nc.vector.tensor_tensor(
                        out=_ap(zt[:, 0:1], 0, [[2 * HD, NGc], [1, 2 * HD]]),
                        in0=_ap(kv, lo * 2 * HD, [[2 * HD, NGc], [1, 2 * HD]]),
                        in1=_ap(p16[:, 0:1], 0, [[0, NGc], [1, 2 * HD]]),
                        op=ALU.mult),
                    lambda: nc.vector.tensor_reduce(
                        out=gl[:],
                        in_=_ap(zt[:, 0:1], 0, [[2 * HD, NGc], [1, 2 * HD]]),
                        axis=AX.X, op=ALU.add),
                    lambda: nc.scalar.activation(out=g16[:], in_=gl[:],
                                                 func=ACTF.Sigmoid, bias=bg),
                    lambda: nc.scalar.activation(
                        out=gb[:],
                        in_=_ap(g16[:, 0:1], 0, [[1, NGc], [0, HD]]),
                        func=ACTF.Copy),
                    # rst += gate * (skip - rst)
                    lambda: nc.vector.tensor_tensor(
                        out=dif3, in0=sksl, in1=rstf, op=ALU.subtract),
                    lambda: nc.vector.tensor_tensor(
                        out=dif[:], in0=dif[:], in1=gb[:], op=ALU.mult),
                    lambda: nc.vector.tensor_tensor(
                        out=rstf, in0=rstf, in1=dif3, op=ALU.add),
                    # LayerNorm
                    lambda: nc.vector.tensor_reduce(
                        out=mu[:], in_=rstf, axis=AX.X, op=ALU.add),
                    lambda: nc.scalar.activation(
                        out=mub[:],
                        in_=_ap(mu[:, 0:1], 0, [[1, NGc], [0, HD]]),
                        func=ACTF.Copy, scale=1.0 / HD),
                    lambda: nc.vector.tensor_tensor(
                        out=rstf, in0=rstf, in1=mub3, op=ALU.subtract),
                    lambda: nc.vector.tensor_tensor(
                        out=sq3, in0=rstf, in1=rstf, op=ALU.mult),
                    lambda: nc.vector.tensor_reduce(
                        out=vs[:], in_=sq3, axis=AX.X, op=ALU.add),
                    lambda: nc.scalar.activation(out=vs[:], in_=vs[:],
                                                 func=ACTF.Sqrt,
                                                 scale=1.0 / HD, bias=eps_t),
                    lambda: nc.vector.reciprocal(out=vs[:], in_=vs[:]),
                    lambda: nc.scalar.activation(
                        out=rb[:],
                        in_=_ap(vs[:, 0:1], 0, [[1, NGc], [0, HD]]),
                        func=ACTF.Copy),
                    lambda: nc.vector.tensor_tensor(
                        out=rstf, in0=rstf, in1=rb3, op=ALU.mult),
                    lambda: nc.vector.tensor_tensor(
                        out=rstf, in0=rstf,
                        in1=_ap(p16[:, 0:1], 2 * HD, [[0, NGc], [1, HD]]),
                        op=ALU.mult),
                    lambda: nc.vector.tensor_tensor(
                        out=rstf, in0=rstf,
                        in1=_ap(p16[:, 0:1], 3 * HD, [[0, NGc], [1, HD]]),
                        op=ALU.add),
                    lambda: nc.scalar.activation(out=rstf, in_=rstf,
                                                 func=ACTF.Prelu, alpha=pa),
                    lambda: nc.sync.dma_start(
                        out=out_d[:, lo * 2 * HD:hi * 2 * HD],
                        in_=ksk[:, lo * 2 * HD:hi * 2 * HD]),
                ]
                return ops

            mid = NG // 2
            opsA = node_ops(0, mid)
            opsB = node_ops(mid, NG)
            for a, b in zip(opsA, opsB):
                a()
                b()

    nc.compile()
    return nc


# ------------------------------------------------------------------- driver

_CACHE = {}


def _get_nc(plan, ncores):
    key = (tuple(plan["K"].tolist()), plan["grid"], plan["totc"], ncores)
    if key not in _CACHE:
        _CACHE[key] = _build_nc(plan, ncores)
    return _CACHE[key]


def _make_inmaps(plan, params, ncores):
    (Wk, bk, Wskip, bskip, Wgate, bgate, ln_gamma, ln_beta, prelu_a) = params
    Wk = _perm_dh(np.asarray(Wk, np.float32))
    bk = _perm_dh(np.asarray(bk, np.float32).reshape(HD))
    Wskip = _perm_dh(np.asarray(Wskip, np.float32))
    bskip = _perm_dh(np.asarray(bskip, np.float32).reshape(HD))
    wks = np.zeros((IN_F + 1, 2 * HD), np.float16)
    wks[:IN_F, 0:HD] = Wskip
    wks[IN_F, 0:HD] = bskip
    wks[:IN_F, HD:] = Wk
    wks[IN_F, HD:] = bk

    wg = np.asarray(Wgate, np.float32).reshape(3 * HD)
    par16 = np.zeros((1, 4 * HD), np.float16)
    par16[0, 0:HD] = _perm_dh(wg[0:HD] + wg[2 * HD:])          # acts on skip
    par16[0, HD:2 * HD] = _perm_dh(wg[HD:2 * HD] - wg[2 * HD:])  # on rst
    par16[0, 2 * HD:3 * HD] = _perm_dh(np.asarray(ln_gamma, np.float32))
    par16[0, 3 * HD:] = _perm_dh(np.asarray(ln_beta, np.float32))
    par32 = np.zeros((1, 4), np.float32)
    par32[0, 0] = np.float32(np.asarray(bgate).reshape(-1)[0])
    par32[0, 1] = np.float32(np.asarray(prelu_a).reshape(-1)[0])
    par32[0, 2] = 1e-5
    par32[0, 3] = -2.0

    in_maps = []
    for c in range(ncores):
        pc = plan["per_core"][c]
        m = dict(featT=plan["featTs"][c], negpad=pc["negpad"], tab=pc["tab"],
                 wks=wks, par16=par16, par32=par32)
        in_maps.append(m)
    return in_maps


def run(q_src, v_src, feat, src, dst, Wk, bk, Wskip, bskip, Wgate, bgate,
        ln_gamma, ln_beta, prelu_a, ncores=NCORES, trace=False):
    plan = _plan(q_src, v_src, feat, src, dst, ncores)
    nc = _get_nc(plan, ncores)
    in_maps = _make_inmaps(
        plan, (Wk, bk, Wskip, bskip, Wgate, bgate, ln_gamma, ln_beta, prelu_a),
        ncores)
    res = run_bass_kernel_spmd(nc, in_maps, core_ids=list(range(ncores)),
                               trace=trace)
    n, npc, ngrp = plan["n"], plan["npc"], plan["ngrp"]
    ndum = plan["ndum"]
    out = np.empty((n, HD), np.float32)
    for c in range(ncores):
        r = res.results[c]["out"]                     # [128, ngrp*128] fp16
        rr = r.reshape(P, ngrp, 2, HD)[:, :, 1, :]    # rst slots
        arr = rr.transpose(1, 0, 2).reshape(-1, HD)[ndum:ndum + npc]
        # undo (d, h) interleave -> (h, d)
        arr = arr.reshape(-1, D, H).transpose(0, 2, 1).reshape(-1, HD)
        out[c * npc + plan["cores"][c]["perm"]] = arr
    return out.astype(np.float32), res, plan, in_maps, nc


def kernel(**inputs):
    out, _, _, _, _ = run(**inputs)
    return out


# revision 11
# speedup vs baseline: 1.2957x; 1.2957x over previous
"""Trainium2 Bass kernel for nn_DenTargetTransformerConv (GNN message passing).

Strategy (graph/data parallel, dst-owner sharding across 8 NeuronCores):
  - Nodes are partitioned by dst-id range; each core owns N/8 nodes and all
    edges whose dst falls in its range. Cores are fully independent (the
    "halo exchange" of src features is materialized host-side as per-core
    compacted per-edge tables; the device streams them contiguously).
  - Per core, own nodes are sorted by in-degree and packed into groups of
    128 (SBUF partition dim). Every node in group g gets K[g] edge slots
    (K[g] = max degree in that group position across all cores, so the 8
    cores share one compiled program).
  - All per-edge data lives in fp16 with (d, h)-interleaved head layout so
    every DVE op is a dense step-1 16-bit op (2x perf mode). The edge table
    is stored partition-major in DRAM, so each run is one big contiguous
    dma_start per partition (no gather descriptors).
  - Reductions avoid tensor_reduce (always 1x on DVE) where they are large:
    the D-reduction of scores and the K-reduction of the weighted values are
    log2 trees of 2x tensor_tensor adds. Padded slots carry q=v=0; their
    exp(0-2) contribution to the softmax denominator is removed with a
    host-staged pad-count correction instead of a mask multiply.
  - Runs are software-pipelined (post-exp work of run r issues after the
    pre-exp work of run r+1) so the ACT-engine exp never stalls the DVE.
    The node phase (gate/LayerNorm/PReLU) runs in two interleaved chunks so
    its ACT broadcasts and output DMA overlap DVE work.
"""

import numpy as np

import concourse.bacc as bacc
import concourse.bass as bass
import concourse.tile as tile
from concourse import mybir
from concourse.bass_utils import run_bass_kernel_spmd

F32 = mybir.dt.float32
F16 = mybir.dt.float16
AX = mybir.AxisListType
ALU = mybir.AluOpType
ACTF = mybir.ActivationFunctionType

P = 128
NCORES = 8
HD = 64          # H * D
H, D = 4, 16
IN_F = 64

RUNC = 80        # max slot-columns per merged compute run
# GpSimd tensor ops steal SBUF bandwidth from the DVE (2x-mode ops slow ~2x
# while Q7 runs), so edge-phase offloading to GpSimd is a net loss: keep 0.
GP_MAXRK = 0

# fp16 value the ACT exp produces for a fully-padded slot (exp(0*0.25 - 2))
EXPV = float(np.float32(np.float16(np.exp(-2.0))))


def _perm_dh(m):
    """Permute the last hd axis from (h, d) to (d, h) order."""
    s = m.shape[:-1]
    return m.reshape(*s, H, D).swapaxes(-1, -2).reshape(*s, HD)


# ----------------------------------------------------------------- host prep

def _plan(q_src, v_src, feat, src, dst, ncores):
    n = feat.shape[0]
    npc = n // ncores
    ngrp = (npc + P - 1) // P
    grid = ngrp * P
    ndum = grid - npc

    q2 = _perm_dh(np.asarray(q_src, np.float32).reshape(n, HD))
    v2 = _perm_dh(np.asarray(v_src, np.float32).reshape(n, HD))
    qv = np.concatenate([q2, v2], axis=1).astype(np.float16)   # [n, 128]

    src = np.asarray(src).astype(np.int64)
    dst = np.asarray(dst).astype(np.int64)
    order = np.argsort(dst, kind="stable")
    dst_s, src_s = dst[order], src[order]
    bounds = np.searchsorted(dst_s, np.arange(ncores + 1) * npc)

    cores = []
    gmax = np.zeros((ncores, ngrp), np.int64)
    gdegs = []
    for c in range(ncores):
        lo, hi = bounds[c], bounds[c + 1]
        dstL = dst_s[lo:hi] - c * npc          # ascending
        srcL = src_s[lo:hi]
        deg = np.bincount(dstL, minlength=npc)
        starts = np.concatenate([[0], np.cumsum(deg)])
        rank = np.arange(len(dstL)) - starts[dstL]
        perm = np.argsort(deg, kind="stable")  # ascending degree
        pos_of = np.empty(npc, np.int64)
        pos_of[perm] = ndum + np.arange(npc)
        gd = np.zeros(grid, np.int64)
        gd[ndum:] = deg[perm]
        gmax[c] = gd.reshape(ngrp, P).max(1)
        gdegs.append(gd)
        cores.append(dict(dstL=dstL, srcL=srcL, rank=rank, perm=perm,
                          pos_of=pos_of))

    K = np.maximum(gmax.max(0), 2)             # shared per-group slot count
    colbase = np.concatenate([[0], np.cumsum(K)]).astype(np.int64)
    totc = int(colbase[-1])

    # Per-core edge tables, partition-major: tab[p, col, :] is the qv row of
    # the edge in slot (group g, partition p, rank k), col = colbase[g] + k.
    # negpad removes the padded slots' exp(-2) from the softmax denominator
    # (and folds in the 1e-9 epsilon).
    per_core = []
    for c in range(ncores):
        cd = cores[c]
        pos_e = cd["pos_of"][cd["dstL"]]       # grid position of each edge
        g_e = pos_e // P
        p_e = pos_e % P
        col_e = colbase[g_e] + cd["rank"]
        tab = np.zeros((P, totc, 2 * HD), np.float16)
        tab[p_e, col_e] = qv[cd["srcL"]]
        npad = (K[None, :] - gdegs[c].reshape(ngrp, P).T)      # [P, ngrp]
        negpad = np.repeat((-npad * EXPV + 1e-9).astype(np.float32),
                           H, axis=1)                          # [P, ngrp*H]
        per_core.append(dict(tab=tab.reshape(P, totc * 2 * HD),
                             negpad=negpad))

    # featT with ones row, per core, grid-permuted: [IN_F+1, grid] fp16
    featTs = []
    feat = np.asarray(feat, np.float32)
    for c in range(ncores):
        ft = np.zeros((IN_F + 1, grid), np.float16)
        ft[IN_F, :] = 1.0
        perm = cores[c]["perm"]
        ft[:IN_F, ndum:] = feat[c * npc + perm].T.astype(np.float16)
        featTs.append(ft)

    # Merge consecutive equal-K groups into runs of <= RUNC slot-columns.
    runs = []
    g = 0
    while g < ngrp:
        k = int(K[g])
        ge = g + 1
        while ge < ngrp and int(K[ge]) == k and (ge - g + 1) * k <= RUNC:
            ge += 1
        runs.append((g, ge, k))
        g = ge

    return dict(n=n, npc=npc, ngrp=ngrp, grid=grid, ndum=ndum, K=K,
                colbase=colbase, totc=totc, runs=runs,
                cores=cores, per_core=per_core, featTs=featTs)


# ------------------------------------------------------------- device build

def _ap(view, off, dims):
    """AP over a tile view's buffer: partition dim kept, free dims replaced."""
    return bass.AP(tensor=view.tensor, offset=view.offset + off,
                   ap=[view.ap[0]] + dims)


def _build_nc(plan, ncores):
    ngrp, totc, runs = plan["ngrp"], plan["totc"], plan["runs"]
    grid = plan["grid"]
    colbase = plan["colbase"]
    NG = ngrp

    nc = bacc.Bacc("TRN2", target_bir_lowering=False, debug=False,
                   num_devices=ncores)

    featT_d = nc.dram_tensor("featT", [IN_F + 1, grid], F16,
                             kind="ExternalInput").ap()
    tab_d = nc.dram_tensor("tab", [P, totc * 2 * HD], F16,
                           kind="ExternalInput").ap()
    negpad_d = nc.dram_tensor("negpad", [P, NG * H], F32,
                              kind="ExternalInput").ap()
    # combined node linear weights: cols 0:64 = Wskip|bskip, 64:128 = Wk|bk
    wks_d = nc.dram_tensor("wks", [IN_F + 1, 2 * HD], F16,
                           kind="ExternalInput").ap()
    # fp16 params: [wg_skip (64) | wg_rst (64) | gamma (64) | beta (64)]
    par16_d = nc.dram_tensor("par16", [1, 4 * HD], F16,
                             kind="ExternalInput").ap()
    # fp32 params: [bgate, prelu_a, ln_eps, -2.0]
    par32_d = nc.dram_tensor("par32", [1, 4], F32, kind="ExternalInput").ap()
    out_d = nc.dram_tensor("out", [P, ngrp * 2 * HD], F16,
                           kind="ExternalOutput").ap()

    GRC = min(GP_MAXRK, RUNC)
    with tile.TileContext(nc) as tc:
        with (
            tc.tile_pool(name="singles", bufs=1) as singles,
            tc.tile_pool(name="psum", bufs=2, space="PSUM") as psum,
            tc.tile_pool(name="qvp", bufs=3) as qvp,
            tc.tile_pool(name="scr", bufs=3) as scr,
            tc.tile_pool(name="t1p", bufs=2) as t1p,
            tc.tile_pool(name="t2p", bufs=2) as t2p,
            tc.tile_pool(name="t3p", bufs=2) as t3p,
            tc.tile_pool(name="exp", bufs=4) as exsp,
            tc.tile_pool(name="qvg", bufs=2) as qvg,
            tc.tile_pool(name="scrg", bufs=2) as scrg,
            tc.tile_pool(name="t1g", bufs=2) as t1g,
            tc.tile_pool(name="t2g", bufs=2) as t2g,
            tc.tile_pool(name="t3g", bufs=2) as t3g,
            tc.tile_pool(name="exg", bufs=8) as exg,
        ):
            # ---- static loads (wks first: matmuls need it + featT chunk)
            wks_sb = singles.tile([IN_F + 1, 2 * HD], F16)
            nc.sync.dma_start(out=wks_sb[:], in_=wks_d[:])
            featT = singles.tile([IN_F + 1, grid], F16)
            FCH = 13 * P
            for f0 in range(0, grid, FCH):
                f1 = min(grid, f0 + FCH)
                nc.sync.dma_start(out=featT[:, f0:f1], in_=featT_d[:, f0:f1])
            p16 = singles.tile([P, 4 * HD], F16)
            nc.gpsimd.dma_start(
                out=p16[:],
                in_=bass.AP(tensor=par16_d.tensor, offset=par16_d.offset,
                            ap=[[0, P], [1, 4 * HD]]))
            p32 = singles.tile([P, 4], F32)
            nc.gpsimd.dma_start(
                out=p32[:],
                in_=bass.AP(tensor=par32_d.tensor, offset=par32_d.offset,
                            ap=[[0, P], [1, 4]]))
            negpad_sb = singles.tile([P, NG * H], F32)
            nc.sync.dma_start(out=negpad_sb[:], in_=negpad_d[:])
            bg = p32[:, 0:1]
            pa = p32[:, 1:2]
            eps_t = p32[:, 2:3]
            nbias = p32[:, 3:4]          # -2.0 shift for exp

            # ksk: per group g, cols [g*128, g*128+64) = skip,
            # [g*128+64, (g+1)*128) = k16 (later overwritten by rst).
            ksk = singles.tile([P, NG * 2 * HD], F16)
            den = singles.tile([P, NG * H], F32)

            # ---- node linears on PE, 4 groups per PSUM bank
            g = 0
            while g < NG:
                nb = min(4, NG - g)
                pk = psum.tile([P, 512], F32, tag="mm")
                for j in range(nb):
                    nc.tensor.matmul(out=pk[:, j * 128:(j + 1) * 128],
                                     lhsT=featT[:, (g + j) * P:(g + j + 1) * P],
                                     rhs=wks_sb[:],
                                     start=True, stop=True)
                nc.scalar.activation(out=ksk[:, g * 128:(g + nb) * 128],
                                     in_=pk[:, :nb * 128], func=ACTF.Copy)
                g += nb

            # ---- edge phase: big runs on DVE (software-pipelined around the
            # ACT exp), small runs on the otherwise-idle GpSimd engine with
            # dedicated pools so the two streams never share buffers.
            def emit_pre(g0, g1, K, eng, pq, ps, p1, p2, p3, pe, cap):
                R = g1 - g0
                RK = R * K
                c0 = int(colbase[g0])
                qv_t = pq.tile([P, cap * 2 * HD], F16, tag="qv")
                nc.sync.dma_start(out=qv_t[:, :RK * 2 * HD],
                                  in_=tab_d[:, c0 * 2 * HD:(c0 + RK) * 2 * HD])
                qv0 = qv_t[:, 0:1]

                # prod[p, (r,k), dh] = q * k_dst  (k16 bcast over k slots)
                prod = ps.tile([P, cap * HD], F16, tag="scr")
                pr0 = prod[:, 0:1]
                eng.tensor_tensor(
                    out=_ap(pr0, 0, [[HD * K, R], [HD, K], [1, HD]]),
                    in0=_ap(qv0, 0, [[2 * HD * K, R], [2 * HD, K], [1, HD]]),
                    in1=_ap(ksk[:, 0:1], g0 * 2 * HD + HD,
                            [[2 * HD, R], [0, K], [1, HD]]),
                    op=ALU.mult)

                # score tree over d: prod [p, rk, d16, h] -> a [p, rk, h]
                t1 = p1.tile([P, cap * 32], F16, tag="t1")
                eng.tensor_tensor(
                    out=_ap(t1[:, 0:1], 0, [[32, RK], [H, 8], [1, H]]),
                    in0=_ap(pr0, 0, [[HD, RK], [H, 8], [1, H]]),
                    in1=_ap(pr0, 32, [[HD, RK], [H, 8], [1, H]]),
                    op=ALU.add)
                t2 = p2.tile([P, cap * 16], F16, tag="t2")
                eng.tensor_tensor(
                    out=_ap(t2[:, 0:1], 0, [[16, RK], [H, 4], [1, H]]),
                    in0=_ap(t1[:, 0:1], 0, [[32, RK], [H, 4], [1, H]]),
                    in1=_ap(t1[:, 0:1], 16, [[32, RK], [H, 4], [1, H]]),
                    op=ALU.add)
                t3 = p3.tile([P, cap * 8], F16, tag="t3")
                eng.tensor_tensor(
                    out=_ap(t3[:, 0:1], 0, [[8, RK], [H, 2], [1, H]]),
                    in0=_ap(t2[:, 0:1], 0, [[16, RK], [H, 2], [1, H]]),
                    in1=_ap(t2[:, 0:1], 8, [[16, RK], [H, 2], [1, H]]),
                    op=ALU.add)
                ex = pe.tile([P, cap * H], F16, tag="ex")
                eng.tensor_tensor(
                    out=_ap(ex[:, 0:1], 0, [[H, RK], [1, H]]),
                    in0=_ap(t3[:, 0:1], 0, [[2 * H, RK], [1, H]]),
                    in1=_ap(t3[:, 0:1], H, [[2 * H, RK], [1, H]]),
                    op=ALU.add)

                # ex = exp(a/4 - 2) (ACT; padded slots give exp(-2), removed
                # from the denominator via negpad)
                exf = ex[:, :RK * H]
                nc.scalar.activation(out=exf, in_=exf, func=ACTF.Exp,
                                     scale=0.25, bias=nbias)
                return qv_t, ex

            def emit_den(g0, g1, K, ex):
                R = g1 - g0
                nc.vector.tensor_reduce(
                    out=_ap(den[:, 0:1], g0 * H, [[H, R], [1, H]]),
                    in_=_ap(ex[:, 0:1], 0, [[K * H, R], [1, H], [H, K]]),
                    axis=AX.X, op=ALU.add)

            def emit_post(g0, g1, K, qv_t, ex, eng, ps, cap):
                R = g1 - g0
                RK = R * K
                qv0 = qv_t[:, 0:1]
                # w[p, rk, d, h] = v * ex (bcast over d)
                w_t = ps.tile([P, cap * HD], F16, tag="scr")
                w0 = w_t[:, 0:1]
                eng.tensor_tensor(
                    out=_ap(w0, 0, [[HD, RK], [H, D], [1, H]]),
                    in0=_ap(qv0, HD, [[2 * HD, RK], [H, D], [1, H]]),
                    in1=_ap(ex[:, 0:1], 0, [[H, RK], [0, D], [1, H]]),
                    op=ALU.mult)

                # agg tree over k -> rst slot of ksk (fp16)
                klen = K
                while klen > 2:
                    h1 = klen // 2
                    eng.tensor_tensor(
                        out=_ap(w0, 0, [[K * HD, R], [HD, h1], [1, HD]]),
                        in0=_ap(w0, 0, [[K * HD, R], [HD, h1], [1, HD]]),
                        in1=_ap(w0, (klen - h1) * HD,
                                [[K * HD, R], [HD, h1], [1, HD]]),
                        op=ALU.add)
                    klen = h1 + (klen & 1)
                eng.tensor_tensor(
                    out=_ap(ksk[:, 0:1], g0 * 2 * HD + HD,
                            [[2 * HD, R], [1, HD]]),
                    in0=_ap(w0, 0, [[K * HD, R], [1, HD]]),
                    in1=_ap(w0, HD, [[K * HD, R], [1, HD]]),
                    op=ALU.add)

            dve_pools = (nc.vector, qvp, scr, t1p, t2p, t3p, exsp, RUNC)
            gp_pools = (nc.gpsimd, qvg, scrg, t1g, t2g, t3g, exg, GRC)
            gp_dens = []
            pend = None
            for (g0, g1, K) in runs:
                R = g1 - g0
                if R * K <= GP_MAXRK:
                    eng, pq, ps, p1, p2, p3, pe, cap = gp_pools
                    qv_t, ex = emit_pre(g0, g1, K, eng, pq, ps, p1, p2, p3,
                                        pe, cap)
                    emit_post(g0, g1, K, qv_t, ex, eng, ps, cap)
                    gp_dens.append((g0, g1, K, ex))
                else:
                    eng, pq, ps, p1, p2, p3, pe, cap = dve_pools
                    pre = emit_pre(g0, g1, K, eng, pq, ps, p1, p2, p3, pe, cap)
                    if pend is not None:
                        emit_den(pend[0], pend[1], pend[2], pend[4])
                        emit_post(*pend, nc.vector, scr, RUNC)
                    pend = (g0, g1, K) + pre
            if pend is not None:
                emit_den(pend[0], pend[1], pend[2], pend[4])
                emit_post(*pend, nc.vector, scr, RUNC)
            for (g0, g1, K, ex) in gp_dens:
                emit_den(g0, g1, K, ex)

            # ---- node phase: two interleaved chunks of groups
            kv = ksk[:, 0:1]

            def node_ops(lo, hi):
                NGc = hi - lo
                dsl = den[:, lo * H:hi * H]
                nsl = negpad_sb[:, lo * H:hi * H]
                d16 = singles.tile([P, NGc * H], F16)
                gl = singles.tile([P, NGc], F32)
                g16 = singles.tile([P, NGc], F16)
                # bc is reused for the three sequential hd-broadcasts
                # (gate, mean, rstd); dif doubles as the square buffer.
                bc = singles.tile([P, NGc * HD], F16)
                gb = bc
                mub = bc
                rb = bc
                dif = singles.tile([P, NGc * HD], F16)
                sq = dif
                mu = singles.tile([P, NGc], F32)
                vs = singles.tile([P, NGc], F32)
                rstf = _ap(kv, lo * 2 * HD + HD, [[2 * HD, NGc], [1, HD]])
                sksl = _ap(kv, lo * 2 * HD, [[2 * HD, NGc], [1, HD]])
                dif3 = dif[:].rearrange("p (c f) -> p c f", f=HD)
                mub3 = mub[:].rearrange("p (c f) -> p c f", f=HD)
                sq3 = sq[:].rearrange("p (c f) -> p c f", f=HD)
                rb3 = rb[:].rearrange("p (c f) -> p c f", f=HD)
                zt = qvp.tile([P, RUNC * 2 * HD], F16, tag="qv")
                ops = [
                    # dinv = 1 / (den - npad*e^-2 + 1e-9), as fp16
                    lambda: nc.vector.tensor_tensor(
                        out=dsl, in0=dsl, in1=nsl, op=ALU.add),
                    lambda: nc.vector.reciprocal(out=dsl, in_=dsl),
                    lambda: nc.scalar.activation(out=d16[:], in_=dsl,
                                                 func=ACTF.Copy),
                    # rst = agg * dinv
                    lambda: nc.vector.tensor_tensor(
                        out=_ap(kv, lo * 2 * HD + HD,
                                [[2 * HD, NGc], [H, D], [1, H]]),
                        in0=_ap(kv, lo * 2 * HD + HD,
                                [[2 * HD, NGc], [H, D], [1, H]]),
                        in1=_ap(d16[:, 0:1], 0, [[H, NGc], [0, D], [1, H]]),
                        op=ALU.mult),
                    # gate logit z = sum over 128 of [skip|rst]*[wgs|wgr]
                    lambda: nc.vector.tensor_tensor(
                        out=_ap(zt[:, 0:1], 0, [[2 * HD, NGc], [1, 2 * HD]]),
                        in0=_ap(kv, lo * 2 * HD, [[2 * HD, NGc], [1, 2 * HD]]),
                        in1=_ap(p16[:, 0:1], 0, [[0, NGc], [1, 2 * HD]]),
                        op=ALU.mult),
                    lambda: nc.vector.tensor_reduce(
                        out=gl[:],
                        in_=_ap(zt[:, 0:1], 0, [[2 * HD, NGc], [1, 2 * HD]]),
                        axis=AX.X, op=ALU.add),
                    lambda: nc.scalar.activation(out=g16[:], in_=gl[:],
                                                 func=ACTF.Sigmoid, bias=bg),
                    lambda: nc.scalar.activation(
                        out=gb[:],
                        in_=_ap(g16[:, 0:1], 0, [[1, NGc], [0, HD]]),
                        func=ACTF.Copy),
                    # rst += gate * (skip - rst)
                    lambda: nc.vector.tensor_tensor(
                        out=dif3, in0=sksl, in1=rstf, op=ALU.subtract),
                    lambda: nc.vector.tensor_tensor(
                        out=dif[:], in0=dif[:], in1=gb[:], op=ALU.mult),
                    lambda: nc.vector.tensor_tensor(
                        out=rstf, in0=rstf, in1=dif3, op=ALU.add),
                    # LayerNorm
                    lambda: nc.vector.tensor_reduce(
                        out=mu[:], in_=rstf, axis=AX.X, op=ALU.add),
                    lambda: nc.scalar.activation(
                        out=mub[:],
                        in_=_ap(mu[:, 0:1], 0, [[1, NGc], [0, HD]]),
                        func=ACTF.Copy, scale=1.0 / HD),
                    lambda: nc.vector.tensor_tensor(
                        out=rstf, in0=rstf, in1=mub3, op=ALU.subtract),
                    lambda: nc.vector.tensor_tensor(
                        out=sq3, in0=rstf, in1=rstf, op=ALU.mult),
                    lambda: nc.vector.tensor_reduce(
                        out=vs[:], in_=sq3, axis=AX.X, op=ALU.add),
                    lambda: nc.scalar.activation(out=vs[:], in_=vs[:],
                                                 func=ACTF.Sqrt,
                                                 scale=1.0 / HD, bias=eps_t),
                    lambda: nc.vector.reciprocal(out=vs[:], in_=vs[:]),
                    lambda: nc.scalar.activation(
                        out=rb[:],
                        in_=_ap(vs[:, 0:1], 0, [[1, NGc], [0, HD]]),
                        func=ACTF.Copy),
                    lambda: nc.vector.tensor_tensor(
                        out=rstf, in0=rstf, in1=rb3, op=ALU.mult),
                    lambda: nc.vector.tensor_tensor(
                        out=rstf, in0=rstf,
                        in1=_ap(p16[:, 0:1], 2 * HD, [[0, NGc], [1, HD]]),
                        op=ALU.mult),
                    lambda: nc.vector.tensor_tensor(
                        out=rstf, in0=rstf,
                        in1=_ap(p16[:, 0:1], 3 * HD, [[0, NGc], [1, HD]]),
                        op=ALU.add),
                    lambda: nc.scalar.activation(out=rstf, in_=rstf,
                                                 func=ACTF.Prelu, alpha=pa),
                    lambda: nc.sync.dma_start(
                        out=out_d[:, lo * 2 * HD:hi * 2 * HD],
                        in_=ksk[:, lo * 2 * HD:hi * 2 * HD]),
                ]
                return ops

            mid = NG // 2
            opsA = node_ops(0, mid)
            opsB = node_ops(mid, NG)
            for a, b in zip(opsA, opsB):
                a()
                b()

    nc.compile()
    return nc


# ------------------------------------------------------------------- driver

_CACHE = {}


def _get_nc(plan, ncores):
    key = (tuple(plan["K"].tolist()), plan["grid"], plan["totc"], ncores)
    if key not in _CACHE:
        _CACHE[key] = _build_nc(plan, ncores)
    return _CACHE[key]


def _make_inmaps(plan, params, ncores):
    (Wk, bk, Wskip, bskip, Wgate, bgate, ln_gamma, ln_beta, prelu_a) = params
    Wk = _perm_dh(np.asarray(Wk, np.float32))
    bk = _perm_dh(np.asarray(bk, np.float32).reshape(HD))
    Wskip = _perm_dh(np.asarray(Wskip, np.float32))
    bskip = _perm_dh(np.asarray(bskip, np.float32).reshape(HD))
    wks = np.zeros((IN_F + 1, 2 * HD), np.float16)
    wks[:IN_F, 0:HD] = Wskip
    wks[IN_F, 0:HD] = bskip
    wks[:IN_F, HD:] = Wk
    wks[IN_F, HD:] = bk

    wg = np.asarray(Wgate, np.float32).reshape(3 * HD)
    par16 = np.zeros((1, 4 * HD), np.float16)
    par16[0, 0:HD] = _perm_dh(wg[0:HD] + wg[2 * HD:])          # acts on skip
    par16[0, HD:2 * HD] = _perm_dh(wg[HD:2 * HD] - wg[2 * HD:])  # on rst
    par16[0, 2 * HD:3 * HD] = _perm_dh(np.asarray(ln_gamma, np.float32))
    par16[0, 3 * HD:] = _perm_dh(np.asarray(ln_beta, np.float32))
    par32 = np.zeros((1, 4), np.float32)
    par32[0, 0] = np.float32(np.asarray(bgate).reshape(-1)[0])
    par32[0, 1] = np.float32(np.asarray(prelu_a).reshape(-1)[0])
    par32[0, 2] = 1e-5
    par32[0, 3] = -2.0

    in_maps = []
    for c in range(ncores):
        pc = plan["per_core"][c]
        m = dict(featT=plan["featTs"][c], negpad=pc["negpad"], tab=pc["tab"],
                 wks=wks, par16=par16, par32=par32)
        in_maps.append(m)
    return in_maps


def run(q_src, v_src, feat, src, dst, Wk, bk, Wskip, bskip, Wgate, bgate,
        ln_gamma, ln_beta, prelu_a, ncores=NCORES, trace=False):
    plan = _plan(q_src, v_src, feat, src, dst, ncores)
    nc = _get_nc(plan, ncores)
    in_maps = _make_inmaps(
        plan, (Wk, bk, Wskip, bskip, Wgate, bgate, ln_gamma, ln_beta, prelu_a),
        ncores)
    res = run_bass_kernel_spmd(nc, in_maps, core_ids=list(range(ncores)),
                               trace=trace)
    n, npc, ngrp = plan["n"], plan["npc"], plan["ngrp"]
    ndum = plan["ndum"]
    out = np.empty((n, HD), np.float32)
    for c in range(ncores):
        r = res.results[c]["out"]                     # [128, ngrp*128] fp16
        rr = r.reshape(P, ngrp, 2, HD)[:, :, 1, :]    # rst slots
        arr = rr.transpose(1, 0, 2).reshape(-1, HD)[ndum:ndum + npc]
        # undo (d, h) interleave -> (h, d)
        arr = arr.reshape(-1, D, H).transpose(0, 2, 1).reshape(-1, HD)
        out[c * npc + plan["cores"][c]["perm"]] = arr
    return out.astype(np.float32), res, plan, in_maps, nc


def kernel(**inputs):
    out, _, _, _, _ = run(**inputs)
    return out


# revision 17
# speedup vs baseline: 1.3199x; 1.0187x over previous
"""Trainium2 Bass kernel for nn_DenTargetTransformerConv (GNN message passing).

Strategy (graph/data parallel, dst-owner sharding across 8 NeuronCores):
  - Nodes are partitioned by dst-id range; each core owns N/8 nodes and all
    edges whose dst falls in its range. Cores are fully independent (the
    "halo exchange" of src features is materialized host-side as per-core
    compacted per-edge tables; the device streams them contiguously).
  - Per core, own nodes are sorted by in-degree and packed into groups of
    128 (SBUF partition dim). Every node in group g gets K[g] edge slots
    (K[g] = max degree in that group position across all cores, so the 8
    cores share one compiled program).
  - All per-edge data lives in fp16 with (d, h)-interleaved head layout so
    every DVE op is a dense step-1 16-bit op (2x perf mode). The edge table
    is stored partition-major in DRAM, so each run is one big contiguous
    dma_start per partition (no gather descriptors).
  - Reductions avoid tensor_reduce (always 1x on DVE) where they are large:
    the D-reduction of scores and the K-reduction of the weighted values are
    log2 trees of 2x tensor_tensor adds. Padded slots carry q=v=0; their
    exp(0-2) contribution to the softmax denominator is removed with a
    host-staged pad-count correction instead of a mask multiply.
  - Runs are software-pipelined (post-exp work of run r issues after the
    pre-exp work of run r+1) so the ACT-engine exp never stalls the DVE.
    The node phase (gate/LayerNorm/PReLU) runs in two interleaved chunks so
    its ACT broadcasts and output DMA overlap DVE work.
"""

import numpy as np

import concourse.bacc as bacc
import concourse.bass as bass
import concourse.tile as tile
from concourse import mybir
from concourse.bass_utils import run_bass_kernel_spmd

F32 = mybir.dt.float32
F16 = mybir.dt.float16
AX = mybir.AxisListType
ALU = mybir.AluOpType
ACTF = mybir.ActivationFunctionType

P = 128
NCORES = 8
HD = 64          # H * D
H, D = 4, 16
IN_F = 64

RUNC = 80        # max slot-columns per merged compute run
# GpSimd tensor ops steal SBUF bandwidth from the DVE (2x-mode ops slow ~2x
# while Q7 runs), so edge-phase offloading to GpSimd is a net loss: keep 0.
GP_MAXRK = 0

# fp16 value the ACT exp produces for a fully-padded slot (exp(0*0.25 - 2))
EXPV = float(np.float32(np.float16(np.exp(-2.0))))


def _perm_dh(m):
    """Permute the last hd axis from (h, d) to (d, h) order."""
    s = m.shape[:-1]
    return m.reshape(*s, H, D).swapaxes(-1, -2).reshape(*s, HD)


# ----------------------------------------------------------------- host prep

def _plan(q_src, v_src, feat, src, dst, ncores):
    n = feat.shape[0]
    npc = n // ncores
    ngrp = (npc + P - 1) // P
    grid = ngrp * P
    ndum = grid - npc

    q2 = _perm_dh(np.asarray(q_src, np.float32).reshape(n, HD))
    v2 = _perm_dh(np.asarray(v_src, np.float32).reshape(n, HD))
    qv = np.concatenate([q2, v2], axis=1).astype(np.float16)   # [n, 128]

    src = np.asarray(src).astype(np.int64)
    dst = np.asarray(dst).astype(np.int64)

    # Deal nodes to cores round-robin in global-degree order so every core
    # sees an identical degree profile (keeps the shared K[g] tight). Node
    # with degree rank i -> core i % ncores, local slot i // ncores.
    deg_all = np.bincount(dst, minlength=n)
    order_nodes = np.argsort(deg_all, kind="stable")
    owner = np.empty(n, np.int64)
    localid = np.empty(n, np.int64)
    owner[order_nodes] = np.arange(n) % ncores
    localid[order_nodes] = np.arange(n) // ncores

    key = owner[dst] * npc + localid[dst]
    es = np.argsort(key, kind="stable")
    src_s, key_s = src[es], key[es]
    bounds = np.searchsorted(key_s, np.arange(ncores + 1) * npc)

    cores = []
    gmax = np.zeros((ncores, ngrp), np.int64)
    gdegs = []
    for c in range(ncores):
        lo, hi = bounds[c], bounds[c + 1]
        dstL = key_s[lo:hi] - c * npc          # local ids, ascending
        srcL = src_s[lo:hi]
        deg = np.bincount(dstL, minlength=npc)  # ascending by construction
        starts = np.concatenate([[0], np.cumsum(deg)])
        rank = np.arange(len(dstL)) - starts[dstL]
        node_of = order_nodes[np.arange(npc) * ncores + c]
        gd = np.zeros(grid, np.int64)
        gd[ndum:] = deg
        gmax[c] = gd.reshape(ngrp, P).max(1)
        gdegs.append(gd)
        cores.append(dict(dstL=dstL, srcL=srcL, rank=rank, node_of=node_of))

    K = np.maximum(gmax.max(0), 2)             # shared per-group slot count

    # Merge adjacent groups into runs of <= RUNC slot-columns, padding the
    # smaller groups up to the run's max K when the padding cost is below
    # the per-run instruction overhead it saves.
    runs = []
    g = 0
    while g < ngrp:
        kmax = int(K[g])
        ge = g + 1
        pad = 0
        while ge < ngrp:
            nk = max(kmax, int(K[ge]))
            npad_new = pad + (nk - kmax) * (ge - g) + (nk - int(K[ge]))
            if (ge - g + 1) * nk > RUNC or npad_new > 6:
                break
            kmax, pad = nk, npad_new
            ge += 1
        runs.append((g, ge, kmax))
        g = ge
    K_eff = np.zeros(ngrp, np.int64)
    for (g0, g1, k) in runs:
        K_eff[g0:g1] = k
    colbase = np.concatenate([[0], np.cumsum(K_eff)]).astype(np.int64)
    totc = int(colbase[-1])

    # Per-core edge tables, partition-major: tab[p, col, :] is the qv row of
    # the edge in slot (group g, partition p, rank k), col = colbase[g] + k.
    # negpad removes the padded slots' exp(-2) from the softmax denominator
    # (and folds in the 1e-9 epsilon).
    per_core = []
    for c in range(ncores):
        cd = cores[c]
        pos_e = ndum + cd["dstL"]              # grid position of each edge
        g_e = pos_e // P
        p_e = pos_e % P
        col_e = colbase[g_e] + cd["rank"]
        tab = np.zeros((P, totc, 2 * HD), np.float16)
        tab[p_e, col_e] = qv[cd["srcL"]]
        npad = (K_eff[None, :] - gdegs[c].reshape(ngrp, P).T)  # [P, ngrp]
        negpad = np.repeat((-npad * EXPV + 1e-9).astype(np.float32),
                           H, axis=1)                          # [P, ngrp*H]
        per_core.append(dict(tab=tab.reshape(P, totc * 2 * HD),
                             negpad=negpad))

    # featT with ones row, per core, grid-permuted: [IN_F+1, grid] fp16
    featTs = []
    feat = np.asarray(feat, np.float32)
    for c in range(ncores):
        ft = np.zeros((IN_F + 1, grid), np.float16)
        ft[IN_F, :] = 1.0
        ft[:IN_F, ndum:] = feat[cores[c]["node_of"]].T.astype(np.float16)
        featTs.append(ft)

    return dict(n=n, npc=npc, ngrp=ngrp, grid=grid, ndum=ndum, K=K_eff,
                colbase=colbase, totc=totc, runs=runs,
                cores=cores, per_core=per_core, featTs=featTs)


# ------------------------------------------------------------- device build

def _ap(view, off, dims):
    """AP over a tile view's buffer: partition dim kept, free dims replaced."""
    return bass.AP(tensor=view.tensor, offset=view.offset + off,
                   ap=[view.ap[0]] + dims)


def _build_nc(plan, ncores):
    ngrp, totc, runs = plan["ngrp"], plan["totc"], plan["runs"]
    grid = plan["grid"]
    colbase = plan["colbase"]
    NG = ngrp

    nc = bacc.Bacc("TRN2", target_bir_lowering=False, debug=False,
                   num_devices=ncores)

    featT_d = nc.dram_tensor("featT", [IN_F + 1, grid], F16,
                             kind="ExternalInput").ap()
    tab_d = nc.dram_tensor("tab", [P, totc * 2 * HD], F16,
                           kind="ExternalInput").ap()
    negpad_d = nc.dram_tensor("negpad", [P, NG * H], F32,
                              kind="ExternalInput").ap()
    # combined node linear weights: cols 0:64 = Wskip|bskip, 64:128 = Wk|bk
    wks_d = nc.dram_tensor("wks", [IN_F + 1, 2 * HD], F16,
                           kind="ExternalInput").ap()
    # fp16 params: [wg_skip (64) | wg_rst (64) | gamma (64) | beta (64)]
    par16_d = nc.dram_tensor("par16", [1, 4 * HD], F16,
                             kind="ExternalInput").ap()
    # fp32 params: [bgate, prelu_a, ln_eps, -2.0]
    par32_d = nc.dram_tensor("par32", [1, 4], F32, kind="ExternalInput").ap()
    out_d = nc.dram_tensor("out", [P, ngrp * 2 * HD], F16,
                           kind="ExternalOutput").ap()

    GRC = min(GP_MAXRK, RUNC)
    with tile.TileContext(nc) as tc:
        with (
            tc.tile_pool(name="singles", bufs=1) as singles,
            tc.tile_pool(name="psum", bufs=2, space="PSUM") as psum,
            tc.tile_pool(name="qvp", bufs=3) as qvp,
            tc.tile_pool(name="scr", bufs=3) as scr,
            tc.tile_pool(name="t1p", bufs=2) as t1p,
            tc.tile_pool(name="t2p", bufs=2) as t2p,
            tc.tile_pool(name="t3p", bufs=2) as t3p,
            tc.tile_pool(name="exp", bufs=4) as exsp,
            tc.tile_pool(name="qvg", bufs=2) as qvg,
            tc.tile_pool(name="scrg", bufs=2) as scrg,
            tc.tile_pool(name="t1g", bufs=2) as t1g,
            tc.tile_pool(name="t2g", bufs=2) as t2g,
            tc.tile_pool(name="t3g", bufs=2) as t3g,
            tc.tile_pool(name="exg", bufs=8) as exg,
        ):
            # ---- static loads (wks first: matmuls need it + featT chunk)
            wks_sb = singles.tile([IN_F + 1, 2 * HD], F16)
            nc.sync.dma_start(out=wks_sb[:], in_=wks_d[:])
            featT = singles.tile([IN_F + 1, grid], F16)
            FCH = 7 * P
            for f0 in range(0, grid, FCH):
                f1 = min(grid, f0 + FCH)
                nc.sync.dma_start(out=featT[:, f0:f1], in_=featT_d[:, f0:f1])
            p16 = singles.tile([P, 4 * HD], F16)
            nc.gpsimd.dma_start(
                out=p16[:],
                in_=bass.AP(tensor=par16_d.tensor, offset=par16_d.offset,
                            ap=[[0, P], [1, 4 * HD]]))
            p32 = singles.tile([P, 4], F32)
            nc.gpsimd.dma_start(
                out=p32[:],
                in_=bass.AP(tensor=par32_d.tensor, offset=par32_d.offset,
                            ap=[[0, P], [1, 4]]))
            negpad_sb = singles.tile([P, NG * H], F32)
            nc.sync.dma_start(out=negpad_sb[:], in_=negpad_d[:])
            bg = p32[:, 0:1]
            pa = p32[:, 1:2]
            eps_t = p32[:, 2:3]
            nbias = p32[:, 3:4]          # -2.0 shift for exp

            # ksk: per group g, cols [g*128, g*128+64) = skip,
            # [g*128+64, (g+1)*128) = k16 (later overwritten by rst).
            ksk = singles.tile([P, NG * 2 * HD], F16)
            den = singles.tile([P, NG * H], F32)

            # ---- node linears on PE, 4 groups per PSUM bank
            g = 0
            while g < NG:
                nb = min(4, NG - g)
                pk = psum.tile([P, 512], F32, tag="mm")
                for j in range(nb):
                    nc.tensor.matmul(out=pk[:, j * 128:(j + 1) * 128],
                                     lhsT=featT[:, (g + j) * P:(g + j + 1) * P],
                                     rhs=wks_sb[:],
                                     start=True, stop=True)
                nc.scalar.activation(out=ksk[:, g * 128:(g + nb) * 128],
                                     in_=pk[:, :nb * 128], func=ACTF.Copy)
                g += nb

            # ---- edge phase: big runs on DVE (software-pipelined around the
            # ACT exp), small runs on the otherwise-idle GpSimd engine with
            # dedicated pools so the two streams never share buffers.
            def emit_pre(g0, g1, K, eng, pq, ps, p1, p2, p3, pe, cap):
                R = g1 - g0
                RK = R * K
                c0 = int(colbase[g0])
                qv_t = pq.tile([P, cap * 2 * HD], F16, tag="qv")
                nc.sync.dma_start(out=qv_t[:, :RK * 2 * HD],
                                  in_=tab_d[:, c0 * 2 * HD:(c0 + RK) * 2 * HD])
                qv0 = qv_t[:, 0:1]

                # prod[p, (r,k), dh] = q * k_dst  (k16 bcast over k slots)
                prod = ps.tile([P, cap * HD], F16, tag="scr")
                pr0 = prod[:, 0:1]
                eng.tensor_tensor(
                    out=_ap(pr0, 0, [[HD * K, R], [HD, K], [1, HD]]),
                    in0=_ap(qv0, 0, [[2 * HD * K, R], [2 * HD, K], [1, HD]]),
                    in1=_ap(ksk[:, 0:1], g0 * 2 * HD + HD,
                            [[2 * HD, R], [0, K], [1, HD]]),
                    op=ALU.mult)

                # score tree over d: prod [p, rk, d16, h] -> a [p, rk, h]
                t1 = p1.tile([P, cap * 32], F16, tag="t1")
                eng.tensor_tensor(
                    out=_ap(t1[:, 0:1], 0, [[32, RK], [H, 8], [1, H]]),
                    in0=_ap(pr0, 0, [[HD, RK], [H, 8], [1, H]]),
                    in1=_ap(pr0, 32, [[HD, RK], [H, 8], [1, H]]),
                    op=ALU.add)
                t2 = p2.tile([P, cap * 16], F16, tag="t2")
                eng.tensor_tensor(
                    out=_ap(t2[:, 0:1], 0, [[16, RK], [H, 4], [1, H]]),
                    in0=_ap(t1[:, 0:1], 0, [[32, RK], [H, 4], [1, H]]),
                    in1=_ap(t1[:, 0:1], 16, [[32, RK], [H, 4], [1, H]]),
                    op=ALU.add)
                t3 = p3.tile([P, cap * 8], F16, tag="t3")
                eng.tensor_tensor(
                    out=_ap(t3[:, 0:1], 0, [[8, RK], [H, 2], [1, H]]),
                    in0=_ap(t2[:, 0:1], 0, [[16, RK], [H, 2], [1, H]]),
                    in1=_ap(t2[:, 0:1], 8, [[16, RK], [H, 2], [1, H]]),
                    op=ALU.add)
                ex = pe.tile([P, cap * H], F16, tag="ex")
                eng.tensor_tensor(
                    out=_ap(ex[:, 0:1], 0, [[H, RK], [1, H]]),
                    in0=_ap(t3[:, 0:1], 0, [[2 * H, RK], [1, H]]),
                    in1=_ap(t3[:, 0:1], H, [[2 * H, RK], [1, H]]),
                    op=ALU.add)

                # ex = exp(a/4 - 2) (ACT; padded slots give exp(-2), removed
                # from the denominator via negpad)
                exf = ex[:, :RK * H]
                nc.scalar.activation(out=exf, in_=exf, func=ACTF.Exp,
                                     scale=0.25, bias=nbias)
                return qv_t, ex

            def emit_den(g0, g1, K, ex):
                R = g1 - g0
                nc.vector.tensor_reduce(
                    out=_ap(den[:, 0:1], g0 * H, [[H, R], [1, H]]),
                    in_=_ap(ex[:, 0:1], 0, [[K * H, R], [1, H], [H, K]]),
                    axis=AX.X, op=ALU.add)

            def emit_post(g0, g1, K, qv_t, ex, eng, ps, cap):
                R = g1 - g0
                RK = R * K
                qv0 = qv_t[:, 0:1]
                # w[p, rk, d, h] = v * ex (bcast over d)
                w_t = ps.tile([P, cap * HD], F16, tag="scr")
                w0 = w_t[:, 0:1]
                eng.tensor_tensor(
                    out=_ap(w0, 0, [[HD, RK], [H, D], [1, H]]),
                    in0=_ap(qv0, HD, [[2 * HD, RK], [H, D], [1, H]]),
                    in1=_ap(ex[:, 0:1], 0, [[H, RK], [0, D], [1, H]]),
                    op=ALU.mult)

                # agg tree over k -> rst slot of ksk (fp16)
                klen = K
                while klen > 2:
                    h1 = klen // 2
                    eng.tensor_tensor(
                        out=_ap(w0, 0, [[K * HD, R], [HD, h1], [1, HD]]),
                        in0=_ap(w0, 0, [[K * HD, R], [HD, h1], [1, HD]]),
                        in1=_ap(w0, (klen - h1) * HD,
                                [[K * HD, R], [HD, h1], [1, HD]]),
                        op=ALU.add)
                    klen = h1 + (klen & 1)
                eng.tensor_tensor(
                    out=_ap(ksk[:, 0:1], g0 * 2 * HD + HD,
                            [[2 * HD, R], [1, HD]]),
                    in0=_ap(w0, 0, [[K * HD, R], [1, HD]]),
                    in1=_ap(w0, HD, [[K * HD, R], [1, HD]]),
                    op=ALU.add)

            dve_pools = (nc.vector, qvp, scr, t1p, t2p, t3p, exsp, RUNC)
            gp_pools = (nc.gpsimd, qvg, scrg, t1g, t2g, t3g, exg, GRC)
            gp_dens = []
            pend = None
            # smallest runs first: the pipeline's first DMA is short, so the
            # DVE starts sooner; DMA streams faster than DVE per column, so
            # later (larger) runs never starve.
            for (g0, g1, K) in sorted(runs, key=lambda r: (r[1] - r[0]) * r[2]):
                R = g1 - g0
                if R * K <= GP_MAXRK:
                    eng, pq, ps, p1, p2, p3, pe, cap = gp_pools
                    qv_t, ex = emit_pre(g0, g1, K, eng, pq, ps, p1, p2, p3,
                                        pe, cap)
                    emit_post(g0, g1, K, qv_t, ex, eng, ps, cap)
                    gp_dens.append((g0, g1, K, ex))
                else:
                    eng, pq, ps, p1, p2, p3, pe, cap = dve_pools
                    pre = emit_pre(g0, g1, K, eng, pq, ps, p1, p2, p3, pe, cap)
                    if pend is not None:
                        emit_den(pend[0], pend[1], pend[2], pend[4])
                        emit_post(*pend, nc.vector, scr, RUNC)
                    pend = (g0, g1, K) + pre
            if pend is not None:
                emit_den(pend[0], pend[1], pend[2], pend[4])
                emit_post(*pend, nc.vector, scr, RUNC)
            for (g0, g1, K, ex) in gp_dens:
                emit_den(g0, g1, K, ex)

            # ---- node phase: two interleaved chunks of groups
            kv = ksk[:, 0:1]

            def node_ops(lo, hi):
                NGc = hi - lo
                dsl = den[:, lo * H:hi * H]
                nsl = negpad_sb[:, lo * H:hi * H]
                d16 = singles.tile([P, NGc * H], F16)
                gl = singles.tile([P, NGc], F32)
                g16 = singles.tile([P, NGc], F16)
                # bc is reused for the three sequential hd-broadcasts
                # (gate, mean, rstd); dif doubles as the square buffer.
                bc = singles.tile([P, NGc * HD], F16)
                gb = bc
                mub = bc
                rb = bc
                dif = singles.tile([P, NGc * HD], F16)
                sq = dif
                mu = singles.tile([P, NGc], F32)
                vs = singles.tile([P, NGc], F32)
                rstf = _ap(kv, lo * 2 * HD + HD, [[2 * HD, NGc], [1, HD]])
                sksl = _ap(kv, lo * 2 * HD, [[2 * HD, NGc], [1, HD]])
                dif3 = dif[:].rearrange("p (c f) -> p c f", f=HD)
                mub3 = mub[:].rearrange("p (c f) -> p c f", f=HD)
                sq3 = sq[:].rearrange("p (c f) -> p c f", f=HD)
                rb3 = rb[:].rearrange("p (c f) -> p c f", f=HD)
                zt = qvp.tile([P, RUNC * 2 * HD], F16, tag="qv")
                ops = [
                    # dinv = 1 / (den - npad*e^-2 + 1e-9), as fp16
                    lambda: nc.vector.tensor_tensor(
                        out=dsl, in0=dsl, in1=nsl, op=ALU.add),
                    lambda: nc.vector.reciprocal(out=dsl, in_=dsl),
                    lambda: nc.scalar.activation(out=d16[:], in_=dsl,
                                                 func=ACTF.Copy),
                    # rst = agg * dinv
                    lambda: nc.vector.tensor_tensor(
                        out=_ap(kv, lo * 2 * HD + HD,
                                [[2 * HD, NGc], [H, D], [1, H]]),
                        in0=_ap(kv, lo * 2 * HD + HD,
                                [[2 * HD, NGc], [H, D], [1, H]]),
                        in1=_ap(d16[:, 0:1], 0, [[H, NGc], [0, D], [1, H]]),
                        op=ALU.mult),
                    # gate logit z = sum over 128 of [skip|rst]*[wgs|wgr]
                    # (mult, then a log2 tree of 2x adds, final level to f32)
                    lambda: nc.vector.tensor_tensor(
                        out=_ap(zt[:, 0:1], 0, [[2 * HD, NGc], [1, 2 * HD]]),
                        in0=_ap(kv, lo * 2 * HD, [[2 * HD, NGc], [1, 2 * HD]]),
                        in1=_ap(p16[:, 0:1], 0, [[0, NGc], [1, 2 * HD]]),
                        op=ALU.mult),
                ] + [
                    (lambda hw: lambda: nc.vector.tensor_tensor(
                        out=_ap(zt[:, 0:1], 0, [[2 * HD, NGc], [1, hw]]),
                        in0=_ap(zt[:, 0:1], 0, [[2 * HD, NGc], [1, hw]]),
                        in1=_ap(zt[:, 0:1], hw, [[2 * HD, NGc], [1, hw]]),
                        op=ALU.add))(hw)
                    for hw in (64, 32, 16, 8, 4, 2)
                ] + [
                    lambda: nc.vector.tensor_tensor(
                        out=gl[:].rearrange("p (c f) -> p c f", f=1),
                        in0=_ap(zt[:, 0:1], 0, [[2 * HD, NGc], [1, 1]]),
                        in1=_ap(zt[:, 0:1], 1, [[2 * HD, NGc], [1, 1]]),
                        op=ALU.add),
                    lambda: nc.scalar.activation(out=g16[:], in_=gl[:],
                                                 func=ACTF.Sigmoid, bias=bg),
                    lambda: nc.scalar.activation(
                        out=gb[:],
                        in_=_ap(g16[:, 0:1], 0, [[1, NGc], [0, HD]]),
                        func=ACTF.Copy),
                    # rst += gate * (skip - rst)
                    lambda: nc.vector.tensor_tensor(
                        out=dif3, in0=sksl, in1=rstf, op=ALU.subtract),
                    lambda: nc.vector.tensor_tensor(
                        out=dif[:], in0=dif[:], in1=gb[:], op=ALU.mult),
                    lambda: nc.vector.tensor_tensor(
                        out=rstf, in0=rstf, in1=dif3, op=ALU.add),
                    # LayerNorm
                    lambda: nc.vector.tensor_reduce(
                        out=mu[:], in_=rstf, axis=AX.X, op=ALU.add),
                    lambda: nc.scalar.activation(
                        out=mub[:],
                        in_=_ap(mu[:, 0:1], 0, [[1, NGc], [0, HD]]),
                        func=ACTF.Copy, scale=1.0 / HD),
                    lambda: nc.vector.tensor_tensor(
                        out=rstf, in0=rstf, in1=mub3, op=ALU.subtract),
                    lambda: nc.vector.tensor_tensor(
                        out=sq3, in0=rstf, in1=rstf, op=ALU.mult),
                    lambda: nc.vector.tensor_reduce(
                        out=vs[:], in_=sq3, axis=AX.X, op=ALU.add),
                    lambda: nc.scalar.activation(out=vs[:], in_=vs[:],
                                                 func=ACTF.Sqrt,
                                                 scale=1.0 / HD, bias=eps_t),
                    lambda: nc.vector.reciprocal(out=vs[:], in_=vs[:]),
                    lambda: nc.scalar.activation(
                        out=rb[:],
                        in_=_ap(vs[:, 0:1], 0, [[1, NGc], [0, HD]]),
                        func=ACTF.Copy),
                    lambda: nc.vector.tensor_tensor(
                        out=rstf, in0=rstf, in1=rb3, op=ALU.mult),
                    lambda: nc.vector.tensor_tensor(
                        out=rstf, in0=rstf,
                        in1=_ap(p16[:, 0:1], 2 * HD, [[0, NGc], [1, HD]]),
                        op=ALU.mult),
                    lambda: nc.vector.tensor_tensor(
                        out=rstf, in0=rstf,
                        in1=_ap(p16[:, 0:1], 3 * HD, [[0, NGc], [1, HD]]),
                        op=ALU.add),
                    lambda: nc.scalar.activation(out=rstf, in_=rstf,
                                                 func=ACTF.Prelu, alpha=pa),
                    lambda: nc.sync.dma_start(
                        out=out_d[:, lo * 2 * HD:hi * 2 * HD],
                        in_=ksk[:, lo * 2 * HD:hi * 2 * HD]),
                ]
                return ops

            c1, c2 = (NG * 2) // 5, (NG * 4) // 5
            chunks = [node_ops(0, c1), node_ops(c1, c2), node_ops(c2, NG)]
            for trio in zip(*chunks):
                for f in trio:
                    f()

    nc.compile()
    return nc


# ------------------------------------------------------------------- driver

_CACHE = {}


def _get_nc(plan, ncores):
    key = (tuple(plan["K"].tolist()), plan["grid"], plan["totc"], ncores)
    if key not in _CACHE:
        _CACHE[key] = _build_nc(plan, ncores)
    return _CACHE[key]


def _make_inmaps(plan, params, ncores):
    (Wk, bk, Wskip, bskip, Wgate, bgate, ln_gamma, ln_beta, prelu_a) = params
    Wk = _perm_dh(np.asarray(Wk, np.float32))
    bk = _perm_dh(np.asarray(bk, np.float32).reshape(HD))
    Wskip = _perm_dh(np.asarray(Wskip, np.float32))
    bskip = _perm_dh(np.asarray(bskip, np.float32).reshape(HD))
    wks = np.zeros((IN_F + 1, 2 * HD), np.float16)
    wks[:IN_F, 0:HD] = Wskip
    wks[IN_F, 0:HD] = bskip
    wks[:IN_F, HD:] = Wk
    wks[IN_F, HD:] = bk

    wg = np.asarray(Wgate, np.float32).reshape(3 * HD)
    par16 = np.zeros((1, 4 * HD), np.float16)
    par16[0, 0:HD] = _perm_dh(wg[0:HD] + wg[2 * HD:])          # acts on skip
    par16[0, HD:2 * HD] = _perm_dh(wg[HD:2 * HD] - wg[2 * HD:])  # on rst
    par16[0, 2 * HD:3 * HD] = _perm_dh(np.asarray(ln_gamma, np.float32))
    par16[0, 3 * HD:] = _perm_dh(np.asarray(ln_beta, np.float32))
    par32 = np.zeros((1, 4), np.float32)
    par32[0, 0] = np.float32(np.asarray(bgate).reshape(-1)[0])
    par32[0, 1] = np.float32(np.asarray(prelu_a).reshape(-1)[0])
    par32[0, 2] = 1e-5
    par32[0, 3] = -2.0

    in_maps = []
    for c in range(ncores):
        pc = plan["per_core"][c]
        m = dict(featT=plan["featTs"][c], negpad=pc["negpad"], tab=pc["tab"],
                 wks=wks, par16=par16, par32=par32)
        in_maps.append(m)
    return in_maps


def run(q_src, v_src, feat, src, dst, Wk, bk, Wskip, bskip, Wgate, bgate,
        ln_gamma, ln_beta, prelu_a, ncores=NCORES, trace=False):
    plan = _plan(q_src, v_src, feat, src, dst, ncores)
    nc = _get_nc(plan, ncores)
    in_maps = _make_inmaps(
        plan, (Wk, bk, Wskip, bskip, Wgate, bgate, ln_gamma, ln_beta, prelu_a),
        ncores)
    res = run_bass_kernel_spmd(nc, in_maps, core_ids=list(range(ncores)),
                               trace=trace)
    n, npc, ngrp = plan["n"], plan["npc"], plan["ngrp"]
    ndum = plan["ndum"]
    out = np.empty((n, HD), np.float32)
    for c in range(ncores):
        r = res.results[c]["out"]                     # [128, ngrp*128] fp16
        rr = r.reshape(P, ngrp, 2, HD)[:, :, 1, :]    # rst slots
        arr = rr.transpose(1, 0, 2).reshape(-1, HD)[ndum:ndum + npc]
        # undo (d, h) interleave -> (h, d)
        arr = arr.reshape(-1, D, H).transpose(0, 2, 1).reshape(-1, HD)
        out[plan["cores"][c]["node_of"]] = arr
    return out.astype(np.float32), res, plan, in_maps, nc


def kernel(**inputs):
    out, _, _, _, _ = run(**inputs)
    return out


# revision 22
# speedup vs baseline: 1.3693x; 1.0375x over previous
"""Trainium2 Bass kernel for nn_DenTargetTransformerConv (GNN message passing).

Strategy (graph/data parallel, dst-owner sharding across 8 NeuronCores):
  - Nodes are partitioned by dst-id range; each core owns N/8 nodes and all
    edges whose dst falls in its range. Cores are fully independent (the
    "halo exchange" of src features is materialized host-side as per-core
    compacted per-edge tables; the device streams them contiguously).
  - Per core, own nodes are sorted by in-degree and packed into groups of
    128 (SBUF partition dim). Every node in group g gets K[g] edge slots
    (K[g] = max degree in that group position across all cores, so the 8
    cores share one compiled program).
  - All per-edge data lives in fp16 with (d, h)-interleaved head layout so
    every DVE op is a dense step-1 16-bit op (2x perf mode). The edge table
    is stored partition-major in DRAM, so each run is one big contiguous
    dma_start per partition (no gather descriptors).
  - Reductions avoid tensor_reduce (always 1x on DVE) where they are large:
    the D-reduction of scores and the K-reduction of the weighted values are
    log2 trees of 2x tensor_tensor adds. Padded slots carry q=v=0; their
    exp(0-2) contribution to the softmax denominator is removed with a
    host-staged pad-count correction instead of a mask multiply.
  - Runs are software-pipelined (post-exp work of run r issues after the
    pre-exp work of run r+1) so the ACT-engine exp never stalls the DVE.
    The node phase (gate/LayerNorm/PReLU) runs in two interleaved chunks so
    its ACT broadcasts and output DMA overlap DVE work.
"""

import numpy as np

import concourse.bacc as bacc
import concourse.bass as bass
import concourse.tile as tile
from concourse import mybir
from concourse.bass_utils import run_bass_kernel_spmd

F32 = mybir.dt.float32
F16 = mybir.dt.float16
AX = mybir.AxisListType
ALU = mybir.AluOpType
ACTF = mybir.ActivationFunctionType

P = 128
NCORES = 8
HD = 64          # H * D
H, D = 4, 16
IN_F = 64

RUNC = 80        # max slot-columns per merged compute run
# GpSimd tensor ops steal SBUF bandwidth from the DVE (2x-mode ops slow ~2x
# while Q7 runs), so edge-phase offloading to GpSimd is a net loss: keep 0.
GP_MAXRK = 0

# fp16 value the ACT exp produces for a fully-padded slot (exp(0*0.25 - 2))
EXPV = float(np.float32(np.float16(np.exp(-2.0))))


def _perm_dh(m):
    """Permute the last hd axis from (h, d) to (d, h) order."""
    s = m.shape[:-1]
    return m.reshape(*s, H, D).swapaxes(-1, -2).reshape(*s, HD)


# ----------------------------------------------------------------- host prep

def _plan(q_src, v_src, feat, src, dst, ncores):
    n = feat.shape[0]
    npc = n // ncores
    ngrp = (npc + P - 1) // P
    grid = ngrp * P
    ndum = grid - npc

    q2 = _perm_dh(np.asarray(q_src, np.float32).reshape(n, HD))
    v2 = _perm_dh(np.asarray(v_src, np.float32).reshape(n, HD))
    qv = np.concatenate([q2, v2], axis=1).astype(np.float16)   # [n, 128]

    src = np.asarray(src).astype(np.int64)
    dst = np.asarray(dst).astype(np.int64)

    # Deal nodes to cores round-robin in global-degree order so every core
    # sees an identical degree profile (keeps the shared K[g] tight). Node
    # with degree rank i -> core i % ncores, local slot i // ncores.
    deg_all = np.bincount(dst, minlength=n)
    order_nodes = np.argsort(deg_all, kind="stable")
    owner = np.empty(n, np.int64)
    localid = np.empty(n, np.int64)
    owner[order_nodes] = np.arange(n) % ncores
    localid[order_nodes] = np.arange(n) // ncores

    key = owner[dst] * npc + localid[dst]
    es = np.argsort(key, kind="stable")
    src_s, key_s = src[es], key[es]
    bounds = np.searchsorted(key_s, np.arange(ncores + 1) * npc)

    cores = []
    gmax = np.zeros((ncores, ngrp), np.int64)
    gdegs = []
    for c in range(ncores):
        lo, hi = bounds[c], bounds[c + 1]
        dstL = key_s[lo:hi] - c * npc          # local ids, ascending
        srcL = src_s[lo:hi]
        deg = np.bincount(dstL, minlength=npc)  # ascending by construction
        starts = np.concatenate([[0], np.cumsum(deg)])
        rank = np.arange(len(dstL)) - starts[dstL]
        node_of = order_nodes[np.arange(npc) * ncores + c]
        gd = np.zeros(grid, np.int64)
        gd[ndum:] = deg
        gmax[c] = gd.reshape(ngrp, P).max(1)
        gdegs.append(gd)
        cores.append(dict(dstL=dstL, srcL=srcL, rank=rank, node_of=node_of))

    K = np.maximum(gmax.max(0), 2)             # shared per-group slot count

    # Merge adjacent groups into runs of <= RUNC slot-columns, padding the
    # smaller groups up to the run's max K when the padding cost is below
    # the per-run instruction overhead it saves.
    runs = []
    g = 0
    while g < ngrp:
        kmax = int(K[g])
        ge = g + 1
        pad = 0
        while ge < ngrp:
            nk = max(kmax, int(K[ge]))
            npad_new = pad + (nk - kmax) * (ge - g) + (nk - int(K[ge]))
            if (ge - g + 1) * nk > RUNC or npad_new > 6:
                break
            kmax, pad = nk, npad_new
            ge += 1
        runs.append((g, ge, kmax))
        g = ge
    K_eff = np.zeros(ngrp, np.int64)
    for (g0, g1, k) in runs:
        K_eff[g0:g1] = k
    colbase = np.concatenate([[0], np.cumsum(K_eff)]).astype(np.int64)
    totc = int(colbase[-1])

    # Per-core edge tables, partition-major: tab[p, col, :] is the qv row of
    # the edge in slot (group g, partition p, rank k), col = colbase[g] + k.
    # negpad removes the padded slots' exp(-2) from the softmax denominator
    # (and folds in the 1e-9 epsilon).
    per_core = []
    for c in range(ncores):
        cd = cores[c]
        pos_e = ndum + cd["dstL"]              # grid position of each edge
        g_e = pos_e // P
        p_e = pos_e % P
        col_e = colbase[g_e] + cd["rank"]
        tab = np.zeros((P, totc, 2 * HD), np.float16)
        tab[p_e, col_e] = qv[cd["srcL"]]
        npad = (K_eff[None, :] - gdegs[c].reshape(ngrp, P).T)  # [P, ngrp]
        negpad = np.repeat((-npad * EXPV + 1e-9).astype(np.float32),
                           H, axis=1)                          # [P, ngrp*H]
        per_core.append(dict(tab=tab.reshape(P, totc * 2 * HD),
                             negpad=negpad))

    # featT with ones row, per core, grid-permuted: [IN_F+1, grid] fp16
    featTs = []
    feat = np.asarray(feat, np.float32)
    for c in range(ncores):
        ft = np.zeros((IN_F + 1, grid), np.float16)
        ft[IN_F, :] = 1.0
        ft[:IN_F, ndum:] = feat[cores[c]["node_of"]].T.astype(np.float16)
        featTs.append(ft)

    return dict(n=n, npc=npc, ngrp=ngrp, grid=grid, ndum=ndum, K=K_eff,
                colbase=colbase, totc=totc, runs=runs,
                cores=cores, per_core=per_core, featTs=featTs)


# ------------------------------------------------------------- device build

def _ap(view, off, dims):
    """AP over a tile view's buffer: partition dim kept, free dims replaced."""
    return bass.AP(tensor=view.tensor, offset=view.offset + off,
                   ap=[view.ap[0]] + dims)


def _build_nc(plan, ncores):
    ngrp, totc, runs = plan["ngrp"], plan["totc"], plan["runs"]
    grid = plan["grid"]
    colbase = plan["colbase"]
    NG = ngrp

    nc = bacc.Bacc("TRN2", target_bir_lowering=False, debug=False,
                   num_devices=ncores)

    featT_d = nc.dram_tensor("featT", [IN_F + 1, grid], F16,
                             kind="ExternalInput").ap()
    tab_d = nc.dram_tensor("tab", [P, totc * 2 * HD], F16,
                           kind="ExternalInput").ap()
    negpad_d = nc.dram_tensor("negpad", [P, NG * H], F32,
                              kind="ExternalInput").ap()
    # combined node linear weights: cols 0:64 = Wskip|bskip, 64:128 = Wk|bk
    wks_d = nc.dram_tensor("wks", [IN_F + 1, 2 * HD], F16,
                           kind="ExternalInput").ap()
    # fp16 params: [wg_skip (64) | wg_rst (64) | gamma (64) | beta (64)]
    par16_d = nc.dram_tensor("par16", [1, 4 * HD], F16,
                             kind="ExternalInput").ap()
    # fp32 params: [bgate, prelu_a, ln_eps, -2.0]
    par32_d = nc.dram_tensor("par32", [1, 4], F32, kind="ExternalInput").ap()
    out_d = nc.dram_tensor("out", [P, ngrp * 2 * HD], F16,
                           kind="ExternalOutput").ap()

    GRC = min(GP_MAXRK, RUNC)
    with tile.TileContext(nc) as tc:
        with (
            tc.tile_pool(name="singles", bufs=1) as singles,
            tc.tile_pool(name="psum", bufs=2, space="PSUM") as psum,
            tc.tile_pool(name="qvp", bufs=3) as qvp,
            tc.tile_pool(name="scr", bufs=3) as scr,
            tc.tile_pool(name="t1p", bufs=2) as t1p,
            tc.tile_pool(name="t2p", bufs=2) as t2p,
            tc.tile_pool(name="t3p", bufs=2) as t3p,
            tc.tile_pool(name="exp", bufs=4) as exsp,
            tc.tile_pool(name="qvg", bufs=2) as qvg,
            tc.tile_pool(name="scrg", bufs=2) as scrg,
            tc.tile_pool(name="t1g", bufs=2) as t1g,
            tc.tile_pool(name="t2g", bufs=2) as t2g,
            tc.tile_pool(name="t3g", bufs=2) as t3g,
            tc.tile_pool(name="exg", bufs=8) as exg,
        ):
            # ---- static loads (wks first: the per-run matmuls need it)
            wks_sb = singles.tile([IN_F + 1, 2 * HD], F16)
            nc.sync.dma_start(out=wks_sb[:], in_=wks_d[:])
            featT = singles.tile([IN_F + 1, grid], F16)
            p16 = singles.tile([P, 4 * HD], F16)
            nc.gpsimd.dma_start(
                out=p16[:],
                in_=bass.AP(tensor=par16_d.tensor, offset=par16_d.offset,
                            ap=[[0, P], [1, 4 * HD]]))
            p32 = singles.tile([P, 4], F32)
            nc.gpsimd.dma_start(
                out=p32[:],
                in_=bass.AP(tensor=par32_d.tensor, offset=par32_d.offset,
                            ap=[[0, P], [1, 4]]))
            negpad_sb = singles.tile([P, NG * H], F32)
            nc.sync.dma_start(out=negpad_sb[:], in_=negpad_d[:])
            bg = p32[:, 0:1]
            pa = p32[:, 1:2]
            eps_t = p32[:, 2:3]
            nbias = p32[:, 3:4]          # -2.0 shift for exp

            # ksk: per group g, cols [g*128, g*128+64) = skip,
            # [g*128+64, (g+1)*128) = k16 (later overwritten by rst).
            ksk = singles.tile([P, NG * 2 * HD], F16)
            den = singles.tile([P, NG * H], F32)

            # per-run node linears on PE (emitted in run order so the first
            # run's k16/skip are ready almost immediately)
            def emit_linears(g0, g1):
                nc.sync.dma_start(out=featT[:, g0 * P:g1 * P],
                                  in_=featT_d[:, g0 * P:g1 * P])
                g = g0
                while g < g1:
                    nb = min(4, g1 - g)
                    pk = psum.tile([P, 512], F32, tag="mm")
                    for j in range(nb):
                        nc.tensor.matmul(
                            out=pk[:, j * 128:(j + 1) * 128],
                            lhsT=featT[:, (g + j) * P:(g + j + 1) * P],
                            rhs=wks_sb[:],
                            start=True, stop=True)
                    nc.scalar.activation(out=ksk[:, g * 128:(g + nb) * 128],
                                         in_=pk[:, :nb * 128], func=ACTF.Copy)
                    g += nb

            # ---- edge phase: big runs on DVE (software-pipelined around the
            # ACT exp), small runs on the otherwise-idle GpSimd engine with
            # dedicated pools so the two streams never share buffers.
            def emit_pre(g0, g1, K, eng, pq, ps, p1, p2, p3, pe, cap):
                R = g1 - g0
                RK = R * K
                c0 = int(colbase[g0])
                emit_linears(g0, g1)
                qv_t = pq.tile([P, cap * 2 * HD], F16, tag="qv")
                nc.sync.dma_start(out=qv_t[:, :RK * 2 * HD],
                                  in_=tab_d[:, c0 * 2 * HD:(c0 + RK) * 2 * HD])
                qv0 = qv_t[:, 0:1]

                # prod[p, (r,k), dh] = q * k_dst  (k16 bcast over k slots)
                prod = ps.tile([P, cap * HD], F16, tag="scr")
                pr0 = prod[:, 0:1]
                eng.tensor_tensor(
                    out=_ap(pr0, 0, [[HD * K, R], [HD, K], [1, HD]]),
                    in0=_ap(qv0, 0, [[2 * HD * K, R], [2 * HD, K], [1, HD]]),
                    in1=_ap(ksk[:, 0:1], g0 * 2 * HD + HD,
                            [[2 * HD, R], [0, K], [1, HD]]),
                    op=ALU.mult)

                # score tree over d: prod [p, rk, d16, h] -> a [p, rk, h]
                t1 = p1.tile([P, cap * 32], F16, tag="t1")
                eng.tensor_tensor(
                    out=_ap(t1[:, 0:1], 0, [[32, RK], [H, 8], [1, H]]),
                    in0=_ap(pr0, 0, [[HD, RK], [H, 8], [1, H]]),
                    in1=_ap(pr0, 32, [[HD, RK], [H, 8], [1, H]]),
                    op=ALU.add)
                t2 = p2.tile([P, cap * 16], F16, tag="t2")
                eng.tensor_tensor(
                    out=_ap(t2[:, 0:1], 0, [[16, RK], [H, 4], [1, H]]),
                    in0=_ap(t1[:, 0:1], 0, [[32, RK], [H, 4], [1, H]]),
                    in1=_ap(t1[:, 0:1], 16, [[32, RK], [H, 4], [1, H]]),
                    op=ALU.add)
                t3 = p3.tile([P, cap * 8], F16, tag="t3")
                eng.tensor_tensor(
                    out=_ap(t3[:, 0:1], 0, [[8, RK], [H, 2], [1, H]]),
                    in0=_ap(t2[:, 0:1], 0, [[16, RK], [H, 2], [1, H]]),
                    in1=_ap(t2[:, 0:1], 8, [[16, RK], [H, 2], [1, H]]),
                    op=ALU.add)
                ex = pe.tile([P, cap * H], F16, tag="ex")
                eng.tensor_tensor(
                    out=_ap(ex[:, 0:1], 0, [[H, RK], [1, H]]),
                    in0=_ap(t3[:, 0:1], 0, [[2 * H, RK], [1, H]]),
                    in1=_ap(t3[:, 0:1], H, [[2 * H, RK], [1, H]]),
                    op=ALU.add)

                # ex = exp(a/4 - 2) (ACT; padded slots give exp(-2), removed
                # from the denominator via negpad)
                exf = ex[:, :RK * H]
                nc.scalar.activation(out=exf, in_=exf, func=ACTF.Exp,
                                     scale=0.25, bias=nbias)
                return qv_t, ex

            def emit_den(g0, g1, K, ex):
                R = g1 - g0
                nc.vector.tensor_reduce(
                    out=_ap(den[:, 0:1], g0 * H, [[H, R], [1, H]]),
                    in_=_ap(ex[:, 0:1], 0, [[K * H, R], [1, H], [H, K]]),
                    axis=AX.X, op=ALU.add)

            def emit_post(g0, g1, K, qv_t, ex, eng, ps, cap):
                R = g1 - g0
                RK = R * K
                qv0 = qv_t[:, 0:1]
                # w[p, rk, d, h] = v * ex (bcast over d)
                w_t = ps.tile([P, cap * HD], F16, tag="scr")
                w0 = w_t[:, 0:1]
                eng.tensor_tensor(
                    out=_ap(w0, 0, [[HD, RK], [H, D], [1, H]]),
                    in0=_ap(qv0, HD, [[2 * HD, RK], [H, D], [1, H]]),
                    in1=_ap(ex[:, 0:1], 0, [[H, RK], [0, D], [1, H]]),
                    op=ALU.mult)

                # agg tree over k -> rst slot of ksk (fp16)
                klen = K
                while klen > 2:
                    h1 = klen // 2
                    eng.tensor_tensor(
                        out=_ap(w0, 0, [[K * HD, R], [HD, h1], [1, HD]]),
                        in0=_ap(w0, 0, [[K * HD, R], [HD, h1], [1, HD]]),
                        in1=_ap(w0, (klen - h1) * HD,
                                [[K * HD, R], [HD, h1], [1, HD]]),
                        op=ALU.add)
                    klen = h1 + (klen & 1)
                eng.tensor_tensor(
                    out=_ap(ksk[:, 0:1], g0 * 2 * HD + HD,
                            [[2 * HD, R], [1, HD]]),
                    in0=_ap(w0, 0, [[K * HD, R], [1, HD]]),
                    in1=_ap(w0, HD, [[K * HD, R], [1, HD]]),
                    op=ALU.add)

            dve_pools = (nc.vector, qvp, scr, t1p, t2p, t3p, exsp, RUNC)
            gp_pools = (nc.gpsimd, qvg, scrg, t1g, t2g, t3g, exg, GRC)
            gp_dens = []
            pend = None
            # smallest runs first: the pipeline's first DMA is short, so the
            # DVE starts sooner; DMA streams faster than DVE per column, so
            # later (larger) runs never starve.
            for (g0, g1, K) in sorted(runs, key=lambda r: (r[1] - r[0]) * r[2]):
                R = g1 - g0
                if R * K <= GP_MAXRK:
                    eng, pq, ps, p1, p2, p3, pe, cap = gp_pools
                    qv_t, ex = emit_pre(g0, g1, K, eng, pq, ps, p1, p2, p3,
                                        pe, cap)
                    emit_post(g0, g1, K, qv_t, ex, eng, ps, cap)
                    gp_dens.append((g0, g1, K, ex))
                else:
                    eng, pq, ps, p1, p2, p3, pe, cap = dve_pools
                    pre = emit_pre(g0, g1, K, eng, pq, ps, p1, p2, p3, pe, cap)
                    if pend is not None:
                        emit_den(pend[0], pend[1], pend[2], pend[4])
                        emit_post(*pend, nc.vector, scr, RUNC)
                    pend = (g0, g1, K) + pre
            if pend is not None:
                emit_den(pend[0], pend[1], pend[2], pend[4])
                emit_post(*pend, nc.vector, scr, RUNC)
            for (g0, g1, K, ex) in gp_dens:
                emit_den(g0, g1, K, ex)

            # ---- node phase: two interleaved chunks of groups
            kv = ksk[:, 0:1]

            def node_ops(lo, hi):
                NGc = hi - lo
                dsl = den[:, lo * H:hi * H]
                nsl = negpad_sb[:, lo * H:hi * H]
                d16 = singles.tile([P, NGc * H], F16)
                gl = singles.tile([P, NGc], F32)
                g16 = singles.tile([P, NGc], F16)
                dif = singles.tile([P, NGc * HD], F16)
                sq = dif                 # dif doubles as the square buffer
                mu = singles.tile([P, NGc], F32)
                vs = singles.tile([P, NGc], F32)
                rstf = _ap(kv, lo * 2 * HD + HD, [[2 * HD, NGc], [1, HD]])
                sksl = _ap(kv, lo * 2 * HD, [[2 * HD, NGc], [1, HD]])
                dif3 = dif[:].rearrange("p (c f) -> p c f", f=HD)
                sq3 = sq[:].rearrange("p (c f) -> p c f", f=HD)
                zt = qvp.tile([P, RUNC * 2 * HD], F16, tag="qv")
                ops = [
                    # dinv = 1 / (den - npad*e^-2 + 1e-9), as fp16
                    lambda: nc.vector.tensor_tensor(
                        out=dsl, in0=dsl, in1=nsl, op=ALU.add),
                    lambda: nc.vector.reciprocal(out=dsl, in_=dsl),
                    lambda: nc.scalar.activation(out=d16[:], in_=dsl,
                                                 func=ACTF.Copy),
                    # rst = agg * dinv
                    lambda: nc.vector.tensor_tensor(
                        out=_ap(kv, lo * 2 * HD + HD,
                                [[2 * HD, NGc], [H, D], [1, H]]),
                        in0=_ap(kv, lo * 2 * HD + HD,
                                [[2 * HD, NGc], [H, D], [1, H]]),
                        in1=_ap(d16[:, 0:1], 0, [[H, NGc], [0, D], [1, H]]),
                        op=ALU.mult),
                    # gate logit z = sum over 128 of [skip|rst]*[wgs|wgr]
                    # (mult, then a log2 tree of 2x adds, final level to f32)
                    lambda: nc.vector.tensor_tensor(
                        out=_ap(zt[:, 0:1], 0, [[2 * HD, NGc], [1, 2 * HD]]),
                        in0=_ap(kv, lo * 2 * HD, [[2 * HD, NGc], [1, 2 * HD]]),
                        in1=_ap(p16[:, 0:1], 0, [[0, NGc], [1, 2 * HD]]),
                        op=ALU.mult),
                ] + [
                    (lambda hw: lambda: nc.vector.tensor_tensor(
                        out=_ap(zt[:, 0:1], 0, [[2 * HD, NGc], [1, hw]]),
                        in0=_ap(zt[:, 0:1], 0, [[2 * HD, NGc], [1, hw]]),
                        in1=_ap(zt[:, 0:1], hw, [[2 * HD, NGc], [1, hw]]),
                        op=ALU.add))(hw)
                    for hw in (64, 32, 16, 8, 4, 2)
                ] + [
                    lambda: nc.vector.tensor_tensor(
                        out=gl[:].rearrange("p (c f) -> p c f", f=1),
                        in0=_ap(zt[:, 0:1], 0, [[2 * HD, NGc], [1, 1]]),
                        in1=_ap(zt[:, 0:1], 1, [[2 * HD, NGc], [1, 1]]),
                        op=ALU.add),
                    lambda: nc.scalar.activation(out=g16[:], in_=gl[:],
                                                 func=ACTF.Sigmoid, bias=bg),
                    # rst += gate * (skip - rst)  (gate bcast directly on DVE)
                    lambda: nc.vector.tensor_tensor(
                        out=dif3, in0=sksl, in1=rstf, op=ALU.subtract),
                    lambda: nc.vector.tensor_tensor(
                        out=dif3, in0=dif3,
                        in1=_ap(g16[:, 0:1], 0, [[1, NGc], [0, HD]]),
                        op=ALU.mult),
                    lambda: nc.vector.tensor_tensor(
                        out=rstf, in0=rstf, in1=dif3, op=ALU.add),
                    # LayerNorm
                    lambda: nc.vector.tensor_reduce(
                        out=mu[:], in_=rstf, axis=AX.X, op=ALU.add),
                    lambda: nc.vector.tensor_scalar(
                        out=mu[:], in0=mu[:], scalar1=1.0 / HD, scalar2=None,
                        op0=ALU.mult),
                    lambda: nc.vector.tensor_tensor(
                        out=rstf, in0=rstf,
                        in1=_ap(mu[:, 0:1], 0, [[1, NGc], [0, HD]]),
                        op=ALU.subtract),
                    lambda: nc.vector.tensor_tensor(
                        out=sq3, in0=rstf, in1=rstf, op=ALU.mult),
                    lambda: nc.vector.tensor_reduce(
                        out=vs[:], in_=sq3, axis=AX.X, op=ALU.add),
                    lambda: nc.scalar.activation(out=vs[:], in_=vs[:],
                                                 func=ACTF.Sqrt,
                                                 scale=1.0 / HD, bias=eps_t),
                    lambda: nc.vector.reciprocal(out=vs[:], in_=vs[:]),
                    lambda: nc.vector.tensor_tensor(
                        out=rstf, in0=rstf,
                        in1=_ap(vs[:, 0:1], 0, [[1, NGc], [0, HD]]),
                        op=ALU.mult),
                    lambda: nc.vector.tensor_tensor(
                        out=rstf, in0=rstf,
                        in1=_ap(p16[:, 0:1], 2 * HD, [[0, NGc], [1, HD]]),
                        op=ALU.mult),
                    lambda: nc.vector.tensor_tensor(
                        out=rstf, in0=rstf,
                        in1=_ap(p16[:, 0:1], 3 * HD, [[0, NGc], [1, HD]]),
                        op=ALU.add),
                    lambda: nc.scalar.activation(out=rstf, in_=rstf,
                                                 func=ACTF.Prelu, alpha=pa),
                    lambda: nc.sync.dma_start(
                        out=out_d[:, lo * 2 * HD:hi * 2 * HD],
                        in_=ksk[:, lo * 2 * HD:hi * 2 * HD]),
                ]
                return ops

            c1, c2 = (NG * 2) // 5, (NG * 4) // 5
            chunks = [node_ops(0, c1), node_ops(c1, c2), node_ops(c2, NG)]
            for trio in zip(*chunks):
                for f in trio:
                    f()

    nc.compile()
    return nc


# ------------------------------------------------------------------- driver

_CACHE = {}


def _get_nc(plan, ncores):
    key = (tuple(plan["K"].tolist()), plan["grid"], plan["totc"], ncores)
    if key not in _CACHE:
        _CACHE[key] = _build_nc(plan, ncores)
    return _CACHE[key]


def _make_inmaps(plan, params, ncores):
    (Wk, bk, Wskip, bskip, Wgate, bgate, ln_gamma, ln_beta, prelu_a) = params
    Wk = _perm_dh(np.asarray(Wk, np.float32))
    bk = _perm_dh(np.asarray(bk, np.float32).reshape(HD))
    Wskip = _perm_dh(np.asarray(Wskip, np.float32))
    bskip = _perm_dh(np.asarray(bskip, np.float32).reshape(HD))
    wks = np.zeros((IN_F + 1, 2 * HD), np.float16)
    wks[:IN_F, 0:HD] = Wskip
    wks[IN_F, 0:HD] = bskip
    wks[:IN_F, HD:] = Wk
    wks[IN_F, HD:] = bk

    wg = np.asarray(Wgate, np.float32).reshape(3 * HD)
    par16 = np.zeros((1, 4 * HD), np.float16)
    par16[0, 0:HD] = _perm_dh(wg[0:HD] + wg[2 * HD:])          # acts on skip
    par16[0, HD:2 * HD] = _perm_dh(wg[HD:2 * HD] - wg[2 * HD:])  # on rst
    par16[0, 2 * HD:3 * HD] = _perm_dh(np.asarray(ln_gamma, np.float32))
    par16[0, 3 * HD:] = _perm_dh(np.asarray(ln_beta, np.float32))
    par32 = np.zeros((1, 4), np.float32)
    par32[0, 0] = np.float32(np.asarray(bgate).reshape(-1)[0])
    par32[0, 1] = np.float32(np.asarray(prelu_a).reshape(-1)[0])
    par32[0, 2] = 1e-5
    par32[0, 3] = -2.0

    in_maps = []
    for c in range(ncores):
        pc = plan["per_core"][c]
        m = dict(featT=plan["featTs"][c], negpad=pc["negpad"], tab=pc["tab"],
                 wks=wks, par16=par16, par32=par32)
        in_maps.append(m)
    return in_maps


def run(q_src, v_src, feat, src, dst, Wk, bk, Wskip, bskip, Wgate, bgate,
        ln_gamma, ln_beta, prelu_a, ncores=NCORES, trace=False):
    plan = _plan(q_src, v_src, feat, src, dst, ncores)
    nc = _get_nc(plan, ncores)
    in_maps = _make_inmaps(
        plan, (Wk, bk, Wskip, bskip, Wgate, bgate, ln_gamma, ln_beta, prelu_a),
        ncores)
    res = run_bass_kernel_spmd(nc, in_maps, core_ids=list(range(ncores)),
                               trace=trace)
    n, npc, ngrp = plan["n"], plan["npc"], plan["ngrp"]
    ndum = plan["ndum"]
    out = np.empty((n, HD), np.float32)
    for c in range(ncores):
        r = res.results[c]["out"]                     # [128, ngrp*128] fp16
        rr = r.reshape(P, ngrp, 2, HD)[:, :, 1, :]    # rst slots
        arr = rr.transpose(1, 0, 2).reshape(-1, HD)[ndum:ndum + npc]
        # undo (d, h) interleave -> (h, d)
        arr = arr.reshape(-1, D, H).transpose(0, 2, 1).reshape(-1, HD)
        out[plan["cores"][c]["node_of"]] = arr
    return out.astype(np.float32), res, plan, in_maps, nc


def kernel(**inputs):
    out, _, _, _, _ = run(**inputs)
    return out
